# revision 66
# baseline (speedup 1.0000x reference)
"""Trainium2 Bass kernel for nn_Decoder_78176994721983 (EGNN-style decoder).

Data-parallel over graphs: 128 graphs x 30 padded nodes, sharded as 16
graphs per NeuronCore across 8 cores. All segment ops are device-local
(edges form a full 30x30 grid inside each graph, so gathers/scatters
become structured broadcast matmuls and grouped free-dim reductions).

Key algorithmic restructuring vs the reference:
  - e_in = [h[row], h[col], radial, edge_attr] @ e_W1 is decomposed into
    node-level pre-products P = h @ e_W1[:256], Q = h @ e_W1[256:512]
    plus an edge-level K=70 matmul whose stationary operand stacks
    [P_g (30 rows), 0, Q_g (30 rows at partition 32), 0, W1_radial,
    W1_edge_attr] and whose moving operand is a static one-hot
    broadcast/tile pattern with radial and edge_attr rows appended.
  - segment_sum over rows = grouped reduction over the inner 30 (col)
    axis of [*, 30a, 30b] views; cnt == 30 exactly (full grid).
  - coord update folds 1/30 into a scalar_tensor_tensor.
"""

import os
import sys

if "/opt/trn_rl_repo" not in sys.path:
    sys.path.insert(0, "/opt/trn_rl_repo")

import numpy as np

B, PAD, LAT, HID = 128, 30, 64, 256
ZIN = LAT + 7  # 71
NCORES = 8
G = B // NCORES            # graphs per core: 16
NL = G * PAD               # nodes per core: 480
EL = G * PAD * PAD         # edges per core: 14400
EG = PAD * PAD             # edges per graph: 900
N_LAYERS = 2

# matmul dtype policy: relaxed fp32 (single-pass PE mode, 4x faster at
# free-dim >= 256) for the big edge matmuls; plain fp32 otherwise.
RELAXED = os.environ.get("KERNEL_F32R", "1") != "0"

_CACHE = {}


# --------------------------------------------------------------------------
# walrus in this container accepts at most ONE sync-wait per instruction;
# Tile emits several. Split extras onto same-engine no-ops just before the
# instruction (same sequencer stream => identical blocking semantics).
def _legalize_waits(nc, mybir):
    n_split = 0
    for f in nc.m.functions:
        for blk in f.blocks:
            insts = list(blk.instructions)
            out = []
            changed = False
            for inst in insts:
                si = inst.sync_info
                if si is not None:
                    waits = list(si.on_wait)
                    if len(waits) > 1:
                        changed = True
                        n_split += 1
                        for j, w in enumerate(waits[:-1]):
                            out.append(
                                mybir.InstNoOp(
                                    name=f"{inst.name}-w{j}",
                                    sync_info=mybir.SyncInfo(
                                        on_wait=[w], on_update=[]
                                    ),
                                    bass_nofuse=True,
                                    engine=inst.engine,
                                )
                            )
                        si.on_wait = waits[-1:]
                out.append(inst)
            if changed:
                blk.instructions = out
    return n_split


def _build_program(legalize=True):
    import concourse.bass as bass
    import concourse.mybir as mybir
    import concourse.tile as tile

    f32 = mybir.dt.float32
    f32r = mybir.dt.float32r
    AF = mybir.ActivationFunctionType
    ALU = mybir.AluOpType
    AX = mybir.AxisListType

    def rx(ap):
        return ap.bitcast(f32r) if RELAXED else ap

    # The BIR verifier requires every producer of an fp32r-matmul operand
    # to emit fp32r-rounded output: write those tensors through fp32r-
    # bitcast views (engines round on the output cast). Non-matmul readers
    # keep plain f32 views of the same bits.
    rw = rx

    nc = bass.Bass()

    def din(name, shape):
        return nc.dram_tensor(name, list(shape), f32, kind="ExternalInput")

    def dout(name, shape):
        return nc.dram_tensor(name, list(shape), f32, kind="ExternalOutput")

    z_d = din("z", [NL, LAT])
    lab_d = din("lab", [NL, 7])
    bc_d = din("BC", [64, EG])
    embw_d = din("embw", [ZIN, HID])
    embb_d = din("embb", [128, 2])
    rp1_d = din("rp1", [ZIN, 15])
    rpb1_d = din("rpb1", [15, 1])
    rp2_d = din("rp2", [15, 3])
    rpb2_d = din("rpb2", [3, 1])
    re1_d = din("re1", [ZIN, 30 * 60])
    reb1_d = din("reb1", [60, 1])
    re2b_d = din("re2b", [61, 4500])
    re2c_d = din("re2c", [61, 4500])
    ones16_d = din("ones16", [1, 128])
    rnb2r_d = din("rnb2r", [1, 5])
    an1_d = din("an1", [ZIN, 30 * 60])
    anb1_d = din("anb1", [60, 1])
    an2_d = din("an2", [60, PAD])
    anb2_d = din("anb2", [PAD, 1])
    idm_d = din("idm", [128, 128])
    rn1_d = din("rn1", [HID, 128])
    big0_d = din("big0", [128, 7448])
    rnb1_d = din("rnb1", [128, 1])
    rn2_d = din("rn2", [128, 5])
    rnb2_d = din("rnb2", [5, 1])

    L = []
    for l in range(N_LAYERS):
        L.append(
            {
                "ehr": din(f"ehr{l}", [HID, HID]),
                "ehc": din(f"ehc{l}", [HID, HID]),
                "wz": din(f"wz{l}", [10, HID]),
                "eb1": din(f"eb1_{l}", [128, 2]),
                "ew2": din(f"ew2_{l}", [HID, HID]),
                "eb2": din(f"eb2_{l}", [128, 2]),
                "cw1": din(f"cw1_{l}", [HID, HID]),
                "cb1": din(f"cb1_{l}", [128, 2]),
                "cw2": din(f"cw2_{l}", [HID, 1]),
                "nw1h": din(f"nw1h_{l}", [HID, HID]),
                "nw1a": din(f"nw1a_{l}", [HID, HID]),
                "nb1": din(f"nb1_{l}", [128, 2]),
                "nw2": din(f"nw2_{l}", [HID, HID]),
                "nb2": din(f"nb2_{l}", [128, 2]),
            }
        )

    hout_d = dout("h_out", [NL, 5])
    coord_d = dout("coord_out", [NL, 3])
    ea_d = dout("ea_out", [EL, 5])
    an_d = dout("an_out", [G, PAD])

    with tile.TileContext(nc) as tc:
        with (
            tc.tile_pool(name="wb", bufs=1) as W,
            tc.tile_pool(name="sb", bufs=2) as SB,
            tc.tile_pool(name="ep", bufs=2, space="PSUM") as EP,
            tc.tile_pool(name="ep2", bufs=5, space="PSUM") as EP2,
            tc.tile_pool(name="wp", bufs=1, space="PSUM") as WP,
        ):
            # ---------------- static loads ----------------
            # Order matters: DMA queues are FIFO, so the tiny inputs that
            # gate the whole dependency tree (z/lab -> z_t transposes ->
            # heads/h/coord) go first, then the weights by first use,
            # with the layer-1 block last.
            idm_t = W.tile([128, 128], f32, tag="idm")
            nc.sync.dma_start(idm_t[:], idm_d.ap())
            z_t = W.tile([ZIN, NL], f32, tag="z_t")
            zls = []
            for c in range(4):
                nsl = slice(c * 120, (c + 1) * 120)
                zl = SB.tile([120, ZIN], f32, tag="zl", name=f"zl{c}")
                nc.sync.dma_start(zl[:, 0:LAT], z_d.ap()[nsl, :])
                nc.scalar.dma_start(zl[:, LAT:ZIN], lab_d.ap()[nsl, :])
                zls.append(zl)
            for c in range(4):
                nsl = slice(c * 120, (c + 1) * 120)
                zps = EP.tile([ZIN, 120], f32, tag="ep", name=f"zps{c}")
                nc.tensor.transpose(zps[:], zls[c][:], idm_t[0:120, 0:120])
                nc.vector.tensor_copy(z_t[:, nsl], zps[:])
            # re2b first: it gates y2_rm -> re6 edge_attr rows -> the very
            # first pre matmul of layer 0 (longest startup pole).
            re2b_t = W.tile([61, 4500], f32, tag="re2b")
            for ci, e in zip(range(3), (nc.scalar, nc.sync, nc.scalar)):
                csl = slice(ci * 1500, (ci + 1) * 1500)
                e.dma_start(rw(re2b_t[:, csl]), rw(re2b_d.ap()[:, csl]))
            big0_t = W.tile([128, 7448], f32, tag="big0")
            nc.sync.dma_start(rw(big0_t[:, 0:1024]), rw(big0_d.ap()[:, 0:1024]))
            nc.scalar.dma_start(
                rw(big0_t[:, 7168:7448]), rw(big0_d.ap()[:, 7168:7448])
            )
            nc.sync.dma_start(
                rw(big0_t[:, 1024:1536]), rw(big0_d.ap()[:, 1024:1536])
            )
            nc.scalar.dma_start(
                rw(big0_t[:, 1536:2048]), rw(big0_d.ap()[:, 1536:2048])
            )
            nc.sync.dma_start(
                rw(big0_t[:, 2048:3584]), rw(big0_d.ap()[:, 2048:3584])
            )
            # zg chunks: zg^T rows [a*71 .. a*71+71) for graph g are just
            # z_t columns g*30+a -> strided slices of z_t, no extra tile.
            zg3 = z_t[:].rearrange("j (g a) -> j a g", a=PAD)
            # Static one-hot broadcast/tile pattern, shared by all graphs.
            # The per-graph radial/edge_attr rows live in a separate
            # [70, 900] tile (rows 64..69) so the pre matmul is
            # K=64 (static) + K=6 (per graph) accumulating in PSUM.
            re6s = []
            for i in range(2):
                r = W.tile([70, EG], f32, tag=f"re6_{i}", name=f"re6s{i}")
                if i == 0:
                    # graph 0's tile gates the first pre matmul: fast
                    # split load on both HWDGE queues
                    nc.sync.dma_start(rw(r[0:32, :]), rw(bc_d.ap()[0:32, :]))
                    nc.scalar.dma_start(rw(r[32:64, :]), rw(bc_d.ap()[32:64, :]))
                else:
                    # off the critical path (first used at graph 1)
                    nc.gpsimd.dma_start(rw(r[0:64, :]), rw(bc_d.ap()))
                re6s.append(r)

            _eng_rot = [nc.sync, nc.scalar, nc.gpsimd]
            _eng_i = [0]

            def _eng():
                e = _eng_rot[_eng_i[0] % len(_eng_rot)]
                _eng_i[0] += 1
                return e

            def wload(d, shape, tag):
                t = W.tile(list(shape), f32, tag=tag)
                _eng().dma_start(t[:], d.ap())
                return t

            embw_t = wload(embw_d, [ZIN, HID], "embw")
            embb_t = wload(embb_d, [128, 2], "embb")
            rp1_t = wload(rp1_d, [ZIN, 15], "rp1")
            rpb1_t = wload(rpb1_d, [15, 1], "rpb1")
            rp2_t = wload(rp2_d, [15, 3], "rp2")
            rpb2_t = wload(rpb2_d, [3, 1], "rpb2")
            re1_t = wload(re1_d, [ZIN, 1800], "re1")
            reb1_t = wload(reb1_d, [60, 1], "reb1")
            re2c_t = W.tile([61, 4500], f32, tag="re2c")
            nc.gpsimd.dma_start(rw(re2c_t[:]), rw(re2c_d.ap()))
            an1_t = wload(an1_d, [ZIN, 1800], "an1")
            anb1_t = wload(anb1_d, [60, 1], "anb1")
            an2_t = wload(an2_d, [60, PAD], "an2")
            anb2_t = wload(anb2_d, [PAD, 1], "anb2")
            rnb1_t = wload(rnb1_d, [128, 1], "rnb1")
            rn2_t = W.tile([128, 5], f32, tag="rn2")
            nc.sync.dma_start(rn2_t[:], rn2_d.ap())
            ones128_t = W.tile([1, 128], f32, tag="ones128")
            nc.scalar.dma_start(ones128_t[:], ones16_d.ap())
            rnb2r_t = W.tile([1, 5], f32, tag="rnb2r")
            nc.scalar.dma_start(rnb2r_t[:], rnb2r_d.ap())
            rn1_t = None  # assigned after the packed load below

            # all [128, x] layer weights packed host-side into one tensor
            # -> a single large contiguous DMA instead of ~32 small ones
            # deferred layer-1 weights (consumed ~halfway through)
            nc.sync.dma_start(
                rw(big0_t[:, 3584:7168]), rw(big0_d.ap()[:, 3584:7168])
            )
            _off = [0]

            def _blk(cols):
                o = _off[0]
                _off[0] += cols
                return big0_t[:, o : o + cols]

            LT = []
            for l in range(N_LAYERS):
                d = L[l]
                t = {}
                for nm in ("ehr", "ehc", "ew2", "cw1", "nw1h", "nw1a", "nw2"):
                    t[nm] = [_blk(HID), _blk(HID)]
                t["wz"] = d["wz"]  # stays in DRAM; DMA'd per graph
                LT.append(t)
            rn1_p = [_blk(128), _blk(128)]
            for l in range(N_LAYERS):
                LT[l]["cw2"] = [_blk(1), _blk(1)]
            for l in range(N_LAYERS):
                for nm in ("eb1", "eb2", "cb1", "nb1", "nb2"):
                    LT[l][nm] = _blk(2)
            rn1_t = rn1_p

            # ---------------- graph-level heads ----------------
            # re_edge hidden: y1 = silu(zg @ W1 + b1), K accumulated in
            # 30 chunks of 71 (one per node slot a).
            y1p = EP.tile([60, G], f32, tag="ep")
            for a in range(PAD):
                nc.tensor.matmul(
                    y1p[:],
                    re1_t[:, a * 60 : (a + 1) * 60],
                    zg3[:, a, :],
                    start=(a == 0),
                    stop=(a == PAD - 1),
                )
            # [y1; ones] so the next matmul folds the bias in
            y1s = SB.tile([61, G], f32, tag="y1s")
            nc.scalar.activation(
                rw(y1s[0:60, :]), y1p[:], AF.Silu, bias=reb1_t[:, 0:1]
            )
            nc.sync.dma_start(rw(y1s[60:61, :]), rw(ones16_d.ap()[:, 0:G]))
            # atom_num hidden
            ay1p = EP.tile([60, G], f32, tag="ep")
            for a in range(PAD):
                nc.tensor.matmul(
                    ay1p[:],
                    an1_t[:, a * 60 : (a + 1) * 60],
                    zg3[:, a, :],
                    start=(a == 0),
                    stop=(a == PAD - 1),
                )
            ay1s = SB.tile([60, G], f32, tag="ay1s")
            nc.scalar.activation(ay1s[:], ay1p[:], AF.Silu, bias=anb1_t[:, 0:1])
            # re_edge out, graph-major with host-permuted columns:
            # y2_rm[g, j*900+k] = edge_attr[g*900+k, j]; bias folded via
            # the ones row of y1s.
            y2_rm = SB.tile([G, 4500], f32, tag="y2_rm", bufs=1)
            for fb in range(9):
                fsl = slice(fb * 500, (fb + 1) * 500)
                y2p = EP.tile([G, 500], f32, tag="ep", name=f"y2p{fb}")
                nc.tensor.matmul(
                    y2p[:], rx(y1s[:]), rx(re2b_t[:, fsl]),
                    start=True, stop=True,
                )
                nc.vector.tensor_copy(y2_rm[:, fsl], y2p[:])
            # edge_attr output: second head pass with unpermuted columns
            # so the DRAM write is fully contiguous (16 descriptors/block)
            ea2d = ea_d.ap().rearrange("e j -> (e j)").rearrange(
                "(g f) -> g f", g=G
            )
            for fb in range(9):
                fsl = slice(fb * 500, (fb + 1) * 500)
                eap = EP.tile([G, 500], f32, tag="ep", name=f"eap{fb}")
                nc.tensor.matmul(
                    eap[:], rx(y1s[:]), rx(re2c_t[:, fsl]),
                    start=True, stop=True,
                )
                eas = SB.tile([G, 500], f32, tag="eas", bufs=2, name=f"eas{fb}")
                nc.vector.tensor_copy(eas[:], eap[:])
                nc.sync.dma_start(ea2d[:, fsl], eas[:])
            # atom_num out
            ayp = EP.tile([PAD, G], f32, tag="ep")
            nc.tensor.matmul(ayp[:], an2_t[:], ay1s[:], start=True, stop=True)
            ay_sb = SB.tile([PAD, G], f32, tag="ay_sb")
            nc.scalar.activation(
                ay_sb[:], ayp[:], AF.Identity, bias=anb2_t[:, 0:1]
            )
            nc.sync.dma_start(
                an_d.ap().rearrange("g k -> k g"), ay_sb[:]
            )


            # ---------------- h / coord init ----------------
            h_cur = []
            for fb in range(2):
                hp = EP.tile([128, NL], f32, tag="ep")
                nc.tensor.matmul(
                    hp[:], embw_t[:, fb * 128 : (fb + 1) * 128], z_t[:],
                    start=True, stop=True,
                )
                ht = SB.tile([128, NL], f32, tag=f"h{fb}")
                nc.scalar.activation(
                    rw(ht[:]), hp[:], AF.Identity, bias=embb_t[:, fb : fb + 1]
                )
                h_cur.append(ht)
            cp1 = EP.tile([15, NL], f32, tag="ep")
            nc.tensor.matmul(cp1[:], rp1_t[:], z_t[:], start=True, stop=True)
            c1s = SB.tile([15, NL], f32, tag="c1s")
            nc.scalar.activation(c1s[:], cp1[:], AF.Silu, bias=rpb1_t[:, 0:1])
            cp2 = EP.tile([3, NL], f32, tag="ep")
            nc.tensor.matmul(cp2[:], rp2_t[:], c1s[:], start=True, stop=True)
            coordT = SB.tile([3, NL], f32, tag="coordT")
            nc.scalar.activation(
                coordT[:], cp2[:], AF.Identity, bias=rpb2_t[:, 0:1]
            )
            # coordS[g, d*30+a] = coord[g*30+a, d]
            coordS = SB.tile([G, 3 * PAD], f32, tag="coordS", bufs=1)
            for d in range(3):
                # shapes differ but iteration orders match: (g, a) vs g*30+a
                nc.sync.dma_start(
                    coordS[:, d * PAD : (d + 1) * PAD],
                    coordT[d : d + 1, :],
                )

            # ---------------- message-passing layers ----------------
            for l in range(N_LAYERS):
                t = LT[l]
                # coordinate differences, stacked graphs on partitions
                cd = []
                for d in range(3):
                    cdt = SB.tile([G, EG], f32, tag=f"cd{d}", bufs=1)
                    ca = coordS[:, d * PAD : (d + 1) * PAD]
                    nc.vector.tensor_sub(
                        cdt[:].rearrange("g (a b) -> g a b", b=PAD),
                        ca.broadcast_to([G, PAD, PAD]),
                        ca[:, None, :].broadcast_to([G, PAD, PAD]),
                    )
                    cd.append(cdt)
                radial = SB.tile([G, EG], f32, tag="radial", bufs=1)
                sqt = SB.tile([G, EG], f32, tag="trans", bufs=1)
                nc.vector.tensor_mul(radial[:], cd[0][:], cd[0][:])
                nc.vector.tensor_mul(sqt[:], cd[1][:], cd[1][:])
                nc.vector.tensor_add(radial[:], radial[:], sqt[:])
                nc.vector.tensor_mul(sqt[:], cd[2][:], cd[2][:])
                nc.vector.tensor_add(radial[:], radial[:], sqt[:])
                agg = [
                    SB.tile([128, NL], f32, tag=f"agg{fb}", bufs=1, name=f"agg{l}_{fb}")
                    for fb in range(2)
                ]
                w_all = SB.tile([G, EG], f32, tag="w_all", bufs=1)

                for g in range(G):
                    ns = slice(g * PAD, (g + 1) * PAD)
                    # P and Q node-level pre-products, both M=30 at (0,0)
                    # (the fp32r ISA check rejects col-offset tile_position,
                    # so Q reaches s70 rows 32..61 via a staging DMA).
                    psP = EP.tile([30, HID], f32, tag="ep", name=f"psP{l}_{g}")
                    for kc in range(2):
                        nc.tensor.matmul(
                            psP[:],
                            rx(h_cur[kc][:, ns]),
                            rx(t["ehr"][kc][:]),
                            start=(kc == 0),
                            stop=(kc == 1),
                        )
                    psQ = EP.tile([30, HID], f32, tag="ep", name=f"psQ{l}_{g}")
                    for kc in range(2):
                        nc.tensor.matmul(
                            psQ[:],
                            rx(h_cur[kc][:, ns]),
                            rx(t["ehc"][kc][:]),
                            start=(kc == 0),
                            stop=(kc == 1),
                        )
                    s70 = SB.tile([70, HID], f32, tag="s70", bufs=4)
                    nc.sync.dma_start(rw(s70[30:32, :]), rw(t["wz"].ap()[0:2, :]))
                    nc.sync.dma_start(rw(s70[62:70, :]), rw(t["wz"].ap()[2:10, :]))
                    nc.vector.tensor_copy(rw(s70[0:30, :]), psP[:])
                    qst = SB.tile([30, HID], f32, tag="qst", bufs=3, name=f"qst{l}_{g}")
                    nc.vector.tensor_copy(rw(qst[:]), psQ[:])
                    nc.sync.dma_start(rw(s70[32:62, :]), rw(qst[:]))
                    # per-graph radial + edge_attr rows at partitions
                    # 64..69 of the static ping-pong tile (BC rows 0..63
                    # were filled once at setup -> single K=70 pre matmul)
                    re6 = re6s[g % 2]
                    nc.sync.dma_start(rw(re6[64:65, :]), rw(radial[g : g + 1, :]))
                    for j in range(5):
                        nc.sync.dma_start(
                            rw(re6[65 + j : 66 + j, :]),
                            rw(y2_rm[g : g + 1, j * 900 : (j + 1) * 900]),
                        )

                    wv_sb = SB.tile([1, EG], f32, tag="wv_sb", bufs=1, name=f"wv_sb{l}_{g}")
                    for hh in range(2):
                        cs = slice(hh * 450, (hh + 1) * 450)
                        # edge MLP stage 1 (pre = e_in @ e_W1)
                        m1 = []
                        for fb in range(2):
                            fs = slice(fb * 128, (fb + 1) * 128)
                            pre = EP2.tile([128, 450], f32, tag="ep2")
                            nc.tensor.matmul(
                                pre[:], rx(s70[:, fs]), rx(re6[:, cs]),
                                start=True, stop=True,
                            )
                            m1t = SB.tile([128, 450], f32, tag=f"m1_{fb}", bufs=4)
                            nc.scalar.activation(
                                rw(m1t[:]), pre[:], AF.Silu,
                                bias=t["eb1"][:, fb : fb + 1],
                            )
                            m1.append(m1t)
                        # edge MLP stage 2 (m = silu(m1 @ e_W2 + b2))
                        mg = []
                        for fb in range(2):
                            fs = slice(fb * 128, (fb + 1) * 128)
                            pre2 = EP2.tile([128, 450], f32, tag="ep2")
                            for kc in range(2):
                                nc.tensor.matmul(
                                    pre2[:], rx(t["ew2"][kc][:, fs]),
                                    rx(m1[kc][:]),
                                    start=(kc == 0), stop=(kc == 1),
                                )
                            mt = SB.tile([128, 450], f32, tag=f"m_{fb}", bufs=4)
                            nc.scalar.activation(
                                rw(mt[:]), pre2[:], AF.Silu,
                                bias=t["eb2"][:, fb : fb + 1],
                            )
                            mg.append(mt)
                        # coord gate hidden
                        mw = []
                        for fb in range(2):
                            fs = slice(fb * 128, (fb + 1) * 128)
                            prew = EP2.tile([128, 450], f32, tag="ep2")
                            for kc in range(2):
                                nc.tensor.matmul(
                                    prew[:], rx(t["cw1"][kc][:, fs]),
                                    rx(mg[kc][:]),
                                    start=(kc == 0), stop=(kc == 1),
                                )
                            mwt = SB.tile([128, 450], f32, tag=f"mw_{fb}")
                            nc.scalar.activation(
                                rw(mwt[:]), prew[:], AF.Silu,
                                bias=t["cb1"][:, fb : fb + 1],
                            )
                            mw.append(mwt)
                        # w = mw @ c_W2 -> [1, 450] (one psum bank per half)
                        wv = WP.tile([1, 450], f32, tag="wp", name=f"wv{l}_{g}_{hh}")
                        for kc in range(2):
                            nc.tensor.matmul(
                                wv[:], rx(t["cw2"][kc][:]), rx(mw[kc][:]),
                                start=(kc == 0), stop=(kc == 1),
                            )
                        nc.vector.tensor_copy(wv_sb[:, cs], wv[:])
                        # agg[f, a] += over this half's 15 a-rows
                        for fb in range(2):
                            with nc.allow_low_precision("f32r agg feeds f32r matmul"):
                                nc.vector.tensor_reduce(
                                    rw(agg[fb][:, g * PAD + hh * 15 : g * PAD + (hh + 1) * 15]),
                                    mg[fb][:].rearrange("p (a b) -> p a b", b=PAD),
                                    axis=AX.X,
                                    op=ALU.add,
                                )
                    # DVE cannot shift partitions: wv_sb staged on partition
                    # 0, DMA moves it to w_all row g.
                    nc.sync.dma_start(w_all[g : g + 1, :], wv_sb[:])

                # coord update: coordS += (sum_b cd*w) / 30
                for d in range(3):
                    trans = SB.tile([G, EG], f32, tag="trans", bufs=1, name=f"trans{l}_{d}")
                    nc.vector.tensor_mul(trans[:], cd[d][:], w_all[:])
                    ssum = SB.tile([G, PAD], f32, tag="ssum", bufs=1, name=f"ssum{l}_{d}")
                    nc.vector.tensor_reduce(
                        ssum[:],
                        trans[:].rearrange("g (a b) -> g a b", b=PAD),
                        axis=AX.X,
                        op=ALU.add,
                    )
                    nc.vector.scalar_tensor_tensor(
                        coordS[:, d * PAD : (d + 1) * PAD],
                        ssum[:],
                        1.0 / PAD,
                        coordS[:, d * PAD : (d + 1) * PAD],
                        op0=ALU.mult,
                        op1=ALU.add,
                    )

                # node update
                s1 = []
                for fb in range(2):
                    hn = EP.tile([128, NL], f32, tag="ep")
                    for kc in range(2):
                        nc.tensor.matmul(
                            hn[:],
                            rx(t["nw1h"][kc][:, fb * 128 : (fb + 1) * 128]),
                            rx(h_cur[kc][:]),
                            start=(kc == 0),
                            stop=False,
                        )
                    for kc in range(2):
                        nc.tensor.matmul(
                            hn[:],
                            rx(t["nw1a"][kc][:, fb * 128 : (fb + 1) * 128]),
                            rx(agg[kc][:]),
                            start=False,
                            stop=(kc == 1),
                        )
                    s1t = SB.tile([128, NL], f32, tag=f"s1_{fb}", bufs=1)
                    nc.scalar.activation(
                        rw(s1t[:]), hn[:], AF.Silu, bias=t["nb1"][:, fb : fb + 1]
                    )
                    s1.append(s1t)
                h_new = []
                for fb in range(2):
                    hp2 = EP.tile([128, NL], f32, tag="ep")
                    for kc in range(2):
                        nc.tensor.matmul(
                            hp2[:],
                            rx(t["nw2"][kc][:, fb * 128 : (fb + 1) * 128]),
                            rx(s1[kc][:]),
                            start=(kc == 0),
                            stop=(kc == 1),
                        )
                    ht = SB.tile([128, NL], f32, tag=f"h{fb}")
                    nc.scalar.activation(
                        rw(ht[:]), hp2[:], AF.Identity, bias=t["nb2"][:, fb : fb + 1]
                    )
                    h_new.append(ht)
                h_cur = h_new

            # ---------------- output heads ----------------
            # coord first so its (scattered) store overlaps the h_out head
            coord3 = coord_d.ap().rearrange("(g a) d -> g d a", a=PAD)
            for d, e in zip(range(3), (nc.sync, nc.scalar, nc.gpsimd)):
                e.dma_start(
                    coord3[:, d, :], coordS[:, d * PAD : (d + 1) * PAD]
                )
            r1p = EP.tile([128, NL], f32, tag="ep")
            for kc in range(2):
                nc.tensor.matmul(
                    r1p[:], rx(rn1_t[kc][:]), rx(h_cur[kc][:]),
                    start=(kc == 0), stop=(kc == 1),
                )
            r1s = SB.tile([128, NL], f32, tag="r1s")
            nc.scalar.activation(r1s[:], r1p[:], AF.Silu, bias=rnb1_t[:, 0:1])
            # node-major blocks: contiguous h_out rows, bias via ones row
            for c in range(4):
                nbl = slice(c * 120, (c + 1) * 120)
                hop = EP.tile([120, 5], f32, tag="ep", name=f"hop{c}")
                nc.tensor.matmul(
                    hop[:], r1s[:, nbl], rn2_t[:],
                    start=True, stop=False,
                )
                nc.tensor.matmul(
                    hop[:], ones128_t[:, 0:120], rnb2r_t[:],
                    start=False, stop=True,
                )
                hos = SB.tile([120, 5], f32, tag="hos", bufs=2, name=f"hos{c}")
                nc.vector.tensor_copy(hos[:], hop[:])
                nc.sync.dma_start(hout_d.ap()[nbl, :], hos[:])

    import concourse.mybir as mybir  # noqa: F811

    if legalize:
        _legalize_waits(nc, mybir)
    return nc


def _prep_weights(params):
    """Host-side weight massaging -> dict of named f32 arrays."""
    def A(x):
        return np.ascontiguousarray(np.asarray(x), dtype=np.float32)

    def col2(b):  # [256] -> [128, 2], column fb = features fb*128..
        return np.ascontiguousarray(A(b).reshape(2, 128).T)

    p = {}
    p["embw"] = A(params["emb_W"])
    p["embb"] = col2(params["emb_b"])
    rp = params["re_pos"]
    p["rp1"] = A(rp["W1"])
    p["rpb1"] = A(rp["b1"]).reshape(15, 1)
    p["rp2"] = A(rp["W2"])
    p["rpb2"] = A(rp["b2"]).reshape(3, 1)
    re = params["re_edge"]
    p["re1"] = np.ascontiguousarray(
        A(re["W1"]).reshape(PAD, ZIN, 60).transpose(1, 0, 2).reshape(ZIN, PAD * 60)
    )
    p["reb1"] = A(re["b1"]).reshape(60, 1)
    W2p = A(re["W2"]).reshape(60, 900, 5).transpose(0, 2, 1).reshape(60, 4500)
    b2p = A(re["b2"]).reshape(900, 5).T.reshape(1, 4500)
    p["re2b"] = np.ascontiguousarray(np.concatenate([W2p, b2p], axis=0))
    p["re2c"] = np.ascontiguousarray(
        np.concatenate([A(re["W2"]), A(re["b2"]).reshape(1, 4500)], axis=0)
    )
    p["ones16"] = np.ones((1, 128), dtype=np.float32)
    an = params["atom_num"]
    p["an1"] = np.ascontiguousarray(
        A(an["W1"]).reshape(PAD, ZIN, 60).transpose(1, 0, 2).reshape(ZIN, PAD * 60)
    )
    p["anb1"] = A(an["b1"]).reshape(60, 1)
    p["an2"] = A(an["W2"])
    p["anb2"] = A(an["b2"]).reshape(PAD, 1)
    rn = params["re_nodes"]
    p["idm"] = np.eye(128, dtype=np.float32)
    p["rn1"] = A(rn["W1"])
    p["rnb1"] = A(rn["b1"]).reshape(128, 1)
    p["rn2"] = A(rn["W2"])
    p["rnb2"] = A(rn["b2"]).reshape(5, 1)
    p["rnb2r"] = A(rn["b2"]).reshape(1, 5)

    # static one-hot broadcast/tile pattern
    bc = np.zeros((64, EG), dtype=np.float32)
    for a in range(PAD):
        bc[a, a * PAD : (a + 1) * PAD] = 1.0
    for b in range(PAD):
        bc[32 + b, b::PAD] = 1.0
    p["BC"] = bc

    blocks = []
    for lp in params["layers"]:
        eW1 = A(lp["e_W1"])
        nW1 = A(lp["n_W1"])
        for arr in (
            eW1[0:256],
            eW1[256:512],
            A(lp["e_W2"]),
            A(lp["c_W1"]),
            nW1[0:256] + nW1[512:768],
            nW1[256:512],
            A(lp["n_W2"]),
        ):
            blocks.append(arr[0:128, :])
            blocks.append(arr[128:256, :])
    blocks.append(A(rn["W1"])[0:128, :])
    blocks.append(A(rn["W1"])[128:256, :])
    for lp in params["layers"]:
        cw2 = A(lp["c_W2"]).reshape(HID, 1)
        blocks.append(cw2[0:128, :])
        blocks.append(cw2[128:256, :])
    for lp in params["layers"]:
        for b in (lp["e_b1"], lp["e_b2"], lp["c_b1"], lp["n_b1"], lp["n_b2"]):
            blocks.append(col2(b))
    p["big0"] = np.ascontiguousarray(np.concatenate(blocks, axis=1))
    assert p["big0"].shape == (128, 7448), p["big0"].shape

    for l, lp in enumerate(params["layers"]):
        eW1 = A(lp["e_W1"])  # [518, 256]
        p[f"ehr{l}"] = np.ascontiguousarray(eW1[0:256])
        p[f"ehc{l}"] = np.ascontiguousarray(eW1[256:512])
        wz = np.zeros((10, HID), dtype=np.float32)
        wz[4] = eW1[512]
        wz[5:10] = eW1[513:518]
        p[f"wz{l}"] = wz
        p[f"eb1_{l}"] = col2(lp["e_b1"])
        p[f"ew2_{l}"] = A(lp["e_W2"])
        p[f"eb2_{l}"] = col2(lp["e_b2"])
        p[f"cw1_{l}"] = A(lp["c_W1"])
        p[f"cb1_{l}"] = col2(lp["c_b1"])
        p[f"cw2_{l}"] = A(lp["c_W2"]).reshape(HID, 1)
        nW1 = A(lp["n_W1"])  # [768, 256]
        p[f"nw1h_{l}"] = np.ascontiguousarray(nW1[0:256] + nW1[512:768])
        p[f"nw1a_{l}"] = np.ascontiguousarray(nW1[256:512])
        p[f"nb1_{l}"] = col2(lp["n_b1"])
        p[f"nw2_{l}"] = A(lp["n_W2"])
        p[f"nb2_{l}"] = col2(lp["n_b2"])
    return p


def _expected_edges():
    i = np.arange(PAD)
    row = np.repeat(i, PAD)
    col = np.tile(i, PAD)
    offs = (np.arange(B) * PAD)[:, None]
    return np.stack(
        [
            (row[None, :] + offs).reshape(-1),
            (col[None, :] + offs).reshape(-1),
        ]
    ).astype(np.int64)


def _get_nc():
    if "nc" not in _CACHE:
        _CACHE["nc"] = _build_program()
    return _CACHE["nc"]


def _install_ntff_hook():
    """antenv.axon_hooks is absent in this image; shim it and register the
    ctypes NTFF profiling hook from trn_agent_boot so trace=True works."""
    import types
    import importlib

    try:
        importlib.import_module("antenv.axon_hooks")
        return
    except ImportError:
        pass
    try:
        from trn_agent_boot.trn_boot import _ntff_profile_via_ctypes
    except ImportError:
        return
    import antenv

    mod = types.ModuleType("antenv.axon_hooks")
    mod._hook = _ntff_profile_via_ctypes("/opt/axon/libaxon_pjrt.so")

    def set_axon_ntff_profile_hook(h):
        mod._hook = h

    def get_axon_ntff_profile_hook():
        return mod._hook

    mod.set_axon_ntff_profile_hook = set_axon_ntff_profile_hook
    mod.get_axon_ntff_profile_hook = get_axon_ntff_profile_hook
    sys.modules["antenv.axon_hooks"] = mod
    antenv.axon_hooks = mod


def kernel(z, label, edges, n_nodes, params, _trace=False):
    from concourse.bass_utils import run_bass_kernel_spmd

    z = np.asarray(z, dtype=np.float32)
    label = np.asarray(label, dtype=np.float32)
    edges_np = np.asarray(edges)
    assert int(n_nodes) == PAD, f"n_nodes must be {PAD}"
    if not np.array_equal(edges_np.astype(np.int64), _expected_edges()):
        raise ValueError("edges do not match the full per-graph grid layout")

    wmap = _prep_weights(params)
    nc = _get_nc()

    in_maps = []
    for c in range(NCORES):
        ns = slice(c * NL, (c + 1) * NL)
        m = dict(wmap)
        m["z"] = np.ascontiguousarray(z[ns])
        m["lab"] = np.ascontiguousarray(label[ns])
        in_maps.append(m)

    kw = {}
    if _trace:
        _install_ntff_hook()
        kw = dict(trace=True, trace_cores=[0])
    res = run_bass_kernel_spmd(nc, in_maps, core_ids=list(range(NCORES)), **kw)

    h_out = np.concatenate([res.results[c]["h_out"] for c in range(NCORES)], 0)
    coord = np.concatenate(
        [res.results[c]["coord_out"] for c in range(NCORES)], 0
    )
    ea = np.concatenate([res.results[c]["ea_out"] for c in range(NCORES)], 0)
    an = np.concatenate([res.results[c]["an_out"] for c in range(NCORES)], 0)
    if _trace:
        return (h_out, coord, ea, an), res
    return h_out, coord, ea, an


# revision 68
# speedup vs baseline: 1.0001x; 1.0001x over previous
"""Trainium2 Bass kernel for nn_Decoder_78176994721983 (EGNN-style decoder).

Data-parallel over graphs: 128 graphs x 30 padded nodes, sharded as 16
graphs per NeuronCore across 8 cores. All segment ops are device-local
(edges form a full 30x30 grid inside each graph, so gathers/scatters
become structured broadcast matmuls and grouped free-dim reductions).

Key algorithmic restructuring vs the reference:
  - e_in = [h[row], h[col], radial, edge_attr] @ e_W1 is decomposed into
    node-level pre-products P = h @ e_W1[:256], Q = h @ e_W1[256:512]
    plus an edge-level K=70 matmul whose stationary operand stacks
    [P_g (30 rows), 0, Q_g (30 rows at partition 32), 0, W1_radial,
    W1_edge_attr] and whose moving operand is a static one-hot
    broadcast/tile pattern with radial and edge_attr rows appended.
  - segment_sum over rows = grouped reduction over the inner 30 (col)
    axis of [*, 30a, 30b] views; cnt == 30 exactly (full grid).
  - coord update folds 1/30 into a scalar_tensor_tensor.
"""

import os
import sys

if "/opt/trn_rl_repo" not in sys.path:
    sys.path.insert(0, "/opt/trn_rl_repo")

import numpy as np

B, PAD, LAT, HID = 128, 30, 64, 256
ZIN = LAT + 7  # 71
NCORES = 8
G = B // NCORES            # graphs per core: 16
NL = G * PAD               # nodes per core: 480
EL = G * PAD * PAD         # edges per core: 14400
EG = PAD * PAD             # edges per graph: 900
N_LAYERS = 2

# matmul dtype policy: relaxed fp32 (single-pass PE mode, 4x faster at
# free-dim >= 256) for the big edge matmuls; plain fp32 otherwise.
RELAXED = os.environ.get("KERNEL_F32R", "1") != "0"

_CACHE = {}


# --------------------------------------------------------------------------
# walrus in this container accepts at most ONE sync-wait per instruction;
# Tile emits several. Split extras onto same-engine no-ops just before the
# instruction (same sequencer stream => identical blocking semantics).
def _legalize_waits(nc, mybir):
    n_split = 0
    for f in nc.m.functions:
        for blk in f.blocks:
            insts = list(blk.instructions)
            out = []
            changed = False
            for inst in insts:
                si = inst.sync_info
                if si is not None:
                    waits = list(si.on_wait)
                    if len(waits) > 1:
                        changed = True
                        n_split += 1
                        for j, w in enumerate(waits[:-1]):
                            out.append(
                                mybir.InstNoOp(
                                    name=f"{inst.name}-w{j}",
                                    sync_info=mybir.SyncInfo(
                                        on_wait=[w], on_update=[]
                                    ),
                                    bass_nofuse=True,
                                    engine=inst.engine,
                                )
                            )
                        si.on_wait = waits[-1:]
                out.append(inst)
            if changed:
                blk.instructions = out
    return n_split


def _build_program(legalize=True):
    import concourse.bass as bass
    import concourse.mybir as mybir
    import concourse.tile as tile

    f32 = mybir.dt.float32
    f32r = mybir.dt.float32r
    AF = mybir.ActivationFunctionType
    ALU = mybir.AluOpType
    AX = mybir.AxisListType

    def rx(ap):
        return ap.bitcast(f32r) if RELAXED else ap

    # The BIR verifier requires every producer of an fp32r-matmul operand
    # to emit fp32r-rounded output: write those tensors through fp32r-
    # bitcast views (engines round on the output cast). Non-matmul readers
    # keep plain f32 views of the same bits.
    rw = rx

    nc = bass.Bass()

    def din(name, shape):
        return nc.dram_tensor(name, list(shape), f32, kind="ExternalInput")

    def dout(name, shape):
        return nc.dram_tensor(name, list(shape), f32, kind="ExternalOutput")

    z_d = din("z", [NL, LAT])
    lab_d = din("lab", [NL, 7])
    bc_d = din("BC", [64, EG])
    embw_d = din("embw", [ZIN, HID])
    embb_d = din("embb", [128, 2])
    rp1_d = din("rp1", [ZIN, 15])
    rpb1_d = din("rpb1", [15, 1])
    rp2_d = din("rp2", [15, 3])
    rpb2_d = din("rpb2", [3, 1])
    re1_d = din("re1", [ZIN, 30 * 60])
    reb1_d = din("reb1", [60, 1])
    re2b_d = din("re2b", [61, 4500])
    re2c_d = din("re2c", [61, 4500])
    ones16_d = din("ones16", [1, 128])
    rnb2r_d = din("rnb2r", [1, 5])
    an1_d = din("an1", [ZIN, 30 * 60])
    anb1_d = din("anb1", [60, 1])
    an2_d = din("an2", [60, PAD])
    anb2_d = din("anb2", [PAD, 1])
    idm_d = din("idm", [128, 128])
    rn1_d = din("rn1", [HID, 128])
    big0_d = din("big0", [128, 7448])
    rnb1_d = din("rnb1", [128, 1])
    rn2_d = din("rn2", [128, 5])
    rnb2_d = din("rnb2", [5, 1])

    L = []
    for l in range(N_LAYERS):
        L.append(
            {
                "ehr": din(f"ehr{l}", [HID, HID]),
                "ehc": din(f"ehc{l}", [HID, HID]),
                "wz": din(f"wz{l}", [10, HID]),
                "eb1": din(f"eb1_{l}", [128, 2]),
                "ew2": din(f"ew2_{l}", [HID, HID]),
                "eb2": din(f"eb2_{l}", [128, 2]),
                "cw1": din(f"cw1_{l}", [HID, HID]),
                "cb1": din(f"cb1_{l}", [128, 2]),
                "cw2": din(f"cw2_{l}", [HID, 1]),
                "nw1h": din(f"nw1h_{l}", [HID, HID]),
                "nw1a": din(f"nw1a_{l}", [HID, HID]),
                "nb1": din(f"nb1_{l}", [128, 2]),
                "nw2": din(f"nw2_{l}", [HID, HID]),
                "nb2": din(f"nb2_{l}", [128, 2]),
            }
        )

    hout_d = dout("h_out", [NL, 5])
    coord_d = dout("coord_out", [NL, 3])
    ea_d = dout("ea_out", [EL, 5])
    an_d = dout("an_out", [G, PAD])

    with tile.TileContext(nc) as tc:
        with (
            tc.tile_pool(name="wb", bufs=1) as W,
            tc.tile_pool(name="sb", bufs=2) as SB,
            tc.tile_pool(name="ep", bufs=2, space="PSUM") as EP,
            tc.tile_pool(name="ep2", bufs=5, space="PSUM") as EP2,
            tc.tile_pool(name="wp", bufs=1, space="PSUM") as WP,
        ):
            # ---------------- static loads ----------------
            # Order matters: DMA queues are FIFO, so the tiny inputs that
            # gate the whole dependency tree (z/lab -> z_t transposes ->
            # heads/h/coord) go first, then the weights by first use,
            # with the layer-1 block last.
            idm_t = W.tile([128, 128], f32, tag="idm")
            nc.sync.dma_start(idm_t[:], idm_d.ap())
            z_t = W.tile([ZIN, NL], f32, tag="z_t")
            zls = []
            for c in range(4):
                nsl = slice(c * 120, (c + 1) * 120)
                zl = SB.tile([120, ZIN], f32, tag="zl", name=f"zl{c}")
                nc.sync.dma_start(zl[:, 0:LAT], z_d.ap()[nsl, :])
                nc.scalar.dma_start(zl[:, LAT:ZIN], lab_d.ap()[nsl, :])
                zls.append(zl)
            for c in range(4):
                nsl = slice(c * 120, (c + 1) * 120)
                zps = EP.tile([ZIN, 120], f32, tag="ep", name=f"zps{c}")
                nc.tensor.transpose(zps[:], zls[c][:], idm_t[0:120, 0:120])
                nc.vector.tensor_copy(z_t[:, nsl], zps[:])
            # re2b first: it gates y2_rm -> re6 edge_attr rows -> the very
            # first pre matmul of layer 0 (longest startup pole).
            re2b_t = W.tile([61, 4500], f32, tag="re2b")
            for ci, e in zip(range(3), (nc.scalar, nc.sync, nc.scalar)):
                csl = slice(ci * 1500, (ci + 1) * 1500)
                e.dma_start(rw(re2b_t[:, csl]), rw(re2b_d.ap()[:, csl]))
            big0_t = W.tile([128, 7448], f32, tag="big0")
            nc.sync.dma_start(rw(big0_t[:, 0:1024]), rw(big0_d.ap()[:, 0:1024]))
            nc.scalar.dma_start(
                rw(big0_t[:, 7168:7448]), rw(big0_d.ap()[:, 7168:7448])
            )
            nc.sync.dma_start(
                rw(big0_t[:, 1024:1536]), rw(big0_d.ap()[:, 1024:1536])
            )
            nc.scalar.dma_start(
                rw(big0_t[:, 1536:2048]), rw(big0_d.ap()[:, 1536:2048])
            )
            nc.sync.dma_start(
                rw(big0_t[:, 2048:3584]), rw(big0_d.ap()[:, 2048:3584])
            )
            # zg chunks: zg^T rows [a*71 .. a*71+71) for graph g are just
            # z_t columns g*30+a -> strided slices of z_t, no extra tile.
            zg3 = z_t[:].rearrange("j (g a) -> j a g", a=PAD)
            # Static one-hot broadcast/tile pattern, shared by all graphs.
            # The per-graph radial/edge_attr rows live in a separate
            # [70, 900] tile (rows 64..69) so the pre matmul is
            # K=64 (static) + K=6 (per graph) accumulating in PSUM.
            re6s = []
            for i in range(2):
                r = W.tile([70, EG], f32, tag=f"re6_{i}", name=f"re6s{i}")
                if i == 0:
                    # graph 0's tile gates the first pre matmul: fast
                    # split load on both HWDGE queues
                    nc.sync.dma_start(rw(r[0:32, :]), rw(bc_d.ap()[0:32, :]))
                    nc.scalar.dma_start(rw(r[32:64, :]), rw(bc_d.ap()[32:64, :]))
                else:
                    # off the critical path (first used at graph 1)
                    nc.gpsimd.dma_start(rw(r[0:64, :]), rw(bc_d.ap()))
                re6s.append(r)

            _eng_rot = [nc.sync, nc.scalar, nc.gpsimd]
            _eng_i = [0]

            def _eng():
                e = _eng_rot[_eng_i[0] % len(_eng_rot)]
                _eng_i[0] += 1
                return e

            def wload(d, shape, tag):
                t = W.tile(list(shape), f32, tag=tag)
                _eng().dma_start(t[:], d.ap())
                return t

            embw_t = wload(embw_d, [ZIN, HID], "embw")
            embb_t = wload(embb_d, [128, 2], "embb")
            rp1_t = wload(rp1_d, [ZIN, 15], "rp1")
            rpb1_t = wload(rpb1_d, [15, 1], "rpb1")
            rp2_t = wload(rp2_d, [15, 3], "rp2")
            rpb2_t = wload(rpb2_d, [3, 1], "rpb2")
            re1_t = wload(re1_d, [ZIN, 1800], "re1")
            reb1_t = wload(reb1_d, [60, 1], "reb1")
            re2c_t = W.tile([61, 4500], f32, tag="re2c")
            nc.gpsimd.dma_start(rw(re2c_t[:]), rw(re2c_d.ap()))
            an1_t = wload(an1_d, [ZIN, 1800], "an1")
            anb1_t = wload(anb1_d, [60, 1], "anb1")
            an2_t = wload(an2_d, [60, PAD], "an2")
            anb2_t = wload(anb2_d, [PAD, 1], "anb2")
            rnb1_t = wload(rnb1_d, [128, 1], "rnb1")
            rn2_t = W.tile([128, 5], f32, tag="rn2")
            nc.sync.dma_start(rn2_t[:], rn2_d.ap())
            ones128_t = W.tile([1, 128], f32, tag="ones128")
            nc.scalar.dma_start(ones128_t[:], ones16_d.ap())
            rnb2r_t = W.tile([1, 5], f32, tag="rnb2r")
            nc.scalar.dma_start(rnb2r_t[:], rnb2r_d.ap())
            rn1_t = None  # assigned after the packed load below

            # all [128, x] layer weights packed host-side into one tensor
            # -> a single large contiguous DMA instead of ~32 small ones
            # deferred layer-1 weights (consumed ~halfway through)
            nc.sync.dma_start(
                rw(big0_t[:, 3584:7168]), rw(big0_d.ap()[:, 3584:7168])
            )
            _off = [0]

            def _blk(cols):
                o = _off[0]
                _off[0] += cols
                return big0_t[:, o : o + cols]

            LT = []
            for l in range(N_LAYERS):
                d = L[l]
                t = {}
                for nm in ("ehr", "ehc", "ew2", "cw1", "nw1h", "nw1a", "nw2"):
                    t[nm] = [_blk(HID), _blk(HID)]
                t["wz"] = d["wz"]  # stays in DRAM; DMA'd per graph
                LT.append(t)
            rn1_p = [_blk(128), _blk(128)]
            for l in range(N_LAYERS):
                LT[l]["cw2"] = [_blk(1), _blk(1)]
            for l in range(N_LAYERS):
                for nm in ("eb1", "eb2", "cb1", "nb1", "nb2"):
                    LT[l][nm] = _blk(2)
            rn1_t = rn1_p

            # ---------------- graph-level heads ----------------
            # re_edge hidden: y1 = silu(zg @ W1 + b1), K accumulated in
            # 30 chunks of 71 (one per node slot a).
            y1p = EP.tile([60, G], f32, tag="ep")
            for a in range(PAD):
                nc.tensor.matmul(
                    y1p[:],
                    re1_t[:, a * 60 : (a + 1) * 60],
                    zg3[:, a, :],
                    start=(a == 0),
                    stop=(a == PAD - 1),
                )
            # [y1; ones] so the next matmul folds the bias in
            y1s = SB.tile([61, G], f32, tag="y1s")
            nc.scalar.activation(
                rw(y1s[0:60, :]), y1p[:], AF.Silu, bias=reb1_t[:, 0:1]
            )
            nc.sync.dma_start(rw(y1s[60:61, :]), rw(ones16_d.ap()[:, 0:G]))
            # atom_num hidden
            ay1p = EP.tile([60, G], f32, tag="ep")
            for a in range(PAD):
                nc.tensor.matmul(
                    ay1p[:],
                    an1_t[:, a * 60 : (a + 1) * 60],
                    zg3[:, a, :],
                    start=(a == 0),
                    stop=(a == PAD - 1),
                )
            ay1s = SB.tile([60, G], f32, tag="ay1s")
            nc.scalar.activation(ay1s[:], ay1p[:], AF.Silu, bias=anb1_t[:, 0:1])
            # re_edge out, graph-major with host-permuted columns:
            # y2_rm[g, j*900+k] = edge_attr[g*900+k, j]; bias folded via
            # the ones row of y1s.
            y2_rm = SB.tile([G, 4500], f32, tag="y2_rm", bufs=1)
            for fb in range(9):
                fsl = slice(fb * 500, (fb + 1) * 500)
                y2p = EP.tile([G, 500], f32, tag="ep", name=f"y2p{fb}")
                nc.tensor.matmul(
                    y2p[:], rx(y1s[:]), rx(re2b_t[:, fsl]),
                    start=True, stop=True,
                )
                nc.vector.tensor_copy(y2_rm[:, fsl], y2p[:])
            # edge_attr output: second head pass with unpermuted columns
            # so the DRAM write is fully contiguous (16 descriptors/block)
            ea2d = ea_d.ap().rearrange("e j -> (e j)").rearrange(
                "(g f) -> g f", g=G
            )
            for fb in range(9):
                fsl = slice(fb * 500, (fb + 1) * 500)
                eap = EP.tile([G, 500], f32, tag="ep", name=f"eap{fb}")
                nc.tensor.matmul(
                    eap[:], rx(y1s[:]), rx(re2c_t[:, fsl]),
                    start=True, stop=True,
                )
                eas = SB.tile([G, 500], f32, tag="eas", bufs=2, name=f"eas{fb}")
                nc.vector.tensor_copy(eas[:], eap[:])
                nc.sync.dma_start(ea2d[:, fsl], eas[:])
            # atom_num out
            ayp = EP.tile([PAD, G], f32, tag="ep")
            nc.tensor.matmul(ayp[:], an2_t[:], ay1s[:], start=True, stop=True)
            ay_sb = SB.tile([PAD, G], f32, tag="ay_sb")
            nc.scalar.activation(
                ay_sb[:], ayp[:], AF.Identity, bias=anb2_t[:, 0:1]
            )
            nc.sync.dma_start(
                an_d.ap().rearrange("g k -> k g"), ay_sb[:]
            )


            # ---------------- h / coord init ----------------
            h_cur = []
            for fb in range(2):
                hp = EP.tile([128, NL], f32, tag="ep")
                nc.tensor.matmul(
                    hp[:], embw_t[:, fb * 128 : (fb + 1) * 128], z_t[:],
                    start=True, stop=True,
                )
                ht = SB.tile([128, NL], f32, tag=f"h{fb}")
                nc.scalar.activation(
                    rw(ht[:]), hp[:], AF.Identity, bias=embb_t[:, fb : fb + 1]
                )
                h_cur.append(ht)
            cp1 = EP.tile([15, NL], f32, tag="ep")
            nc.tensor.matmul(cp1[:], rp1_t[:], z_t[:], start=True, stop=True)
            c1s = SB.tile([15, NL], f32, tag="c1s")
            nc.scalar.activation(c1s[:], cp1[:], AF.Silu, bias=rpb1_t[:, 0:1])
            cp2 = EP.tile([3, NL], f32, tag="ep")
            nc.tensor.matmul(cp2[:], rp2_t[:], c1s[:], start=True, stop=True)
            coordT = SB.tile([3, NL], f32, tag="coordT")
            nc.scalar.activation(
                coordT[:], cp2[:], AF.Identity, bias=rpb2_t[:, 0:1]
            )
            # coordS[g, d*30+a] = coord[g*30+a, d]
            coordS = SB.tile([G, 3 * PAD], f32, tag="coordS", bufs=1)
            for d in range(3):
                # shapes differ but iteration orders match: (g, a) vs g*30+a
                nc.sync.dma_start(
                    coordS[:, d * PAD : (d + 1) * PAD],
                    coordT[d : d + 1, :],
                )

            # ---------------- message-passing layers ----------------
            for l in range(N_LAYERS):
                t = LT[l]
                # coordinate differences, stacked graphs on partitions
                cd = []
                for d in range(3):
                    cdt = SB.tile([G, EG], f32, tag=f"cd{d}", bufs=1)
                    ca = coordS[:, d * PAD : (d + 1) * PAD]
                    nc.vector.tensor_sub(
                        cdt[:].rearrange("g (a b) -> g a b", b=PAD),
                        ca.broadcast_to([G, PAD, PAD]),
                        ca[:, None, :].broadcast_to([G, PAD, PAD]),
                    )
                    cd.append(cdt)
                radial = SB.tile([G, EG], f32, tag="radial", bufs=1)
                sqt = SB.tile([G, EG], f32, tag="trans", bufs=1)
                nc.vector.tensor_mul(radial[:], cd[0][:], cd[0][:])
                nc.vector.tensor_mul(sqt[:], cd[1][:], cd[1][:])
                nc.vector.tensor_add(radial[:], radial[:], sqt[:])
                nc.vector.tensor_mul(sqt[:], cd[2][:], cd[2][:])
                nc.vector.tensor_add(radial[:], radial[:], sqt[:])
                agg = [
                    SB.tile([128, NL], f32, tag=f"agg{fb}", bufs=1, name=f"agg{l}_{fb}")
                    for fb in range(2)
                ]
                w_all = SB.tile([G, EG], f32, tag="w_all", bufs=1)

                for g in range(G):
                    ns = slice(g * PAD, (g + 1) * PAD)
                    # P and Q node-level pre-products, both M=30 at (0,0)
                    # (the fp32r ISA check rejects col-offset tile_position,
                    # so Q reaches s70 rows 32..61 via a staging DMA).
                    psP = EP.tile([30, HID], f32, tag="ep", name=f"psP{l}_{g}")
                    for kc in range(2):
                        nc.tensor.matmul(
                            psP[:],
                            rx(h_cur[kc][:, ns]),
                            rx(t["ehr"][kc][:]),
                            start=(kc == 0),
                            stop=(kc == 1),
                        )
                    psQ = EP.tile([30, HID], f32, tag="ep", name=f"psQ{l}_{g}")
                    for kc in range(2):
                        nc.tensor.matmul(
                            psQ[:],
                            rx(h_cur[kc][:, ns]),
                            rx(t["ehc"][kc][:]),
                            start=(kc == 0),
                            stop=(kc == 1),
                        )
                    s70 = SB.tile([70, HID], f32, tag="s70", bufs=5)
                    nc.sync.dma_start(rw(s70[30:32, :]), rw(t["wz"].ap()[0:2, :]))
                    nc.sync.dma_start(rw(s70[62:70, :]), rw(t["wz"].ap()[2:10, :]))
                    nc.vector.tensor_copy(rw(s70[0:30, :]), psP[:])
                    qst = SB.tile([30, HID], f32, tag="qst", bufs=3, name=f"qst{l}_{g}")
                    nc.vector.tensor_copy(rw(qst[:]), psQ[:])
                    nc.sync.dma_start(rw(s70[32:62, :]), rw(qst[:]))
                    # per-graph radial + edge_attr rows at partitions
                    # 64..69 of the static ping-pong tile (BC rows 0..63
                    # were filled once at setup -> single K=70 pre matmul)
                    re6 = re6s[g % 2]
                    nc.sync.dma_start(rw(re6[64:65, :]), rw(radial[g : g + 1, :]))
                    for j in range(5):
                        nc.sync.dma_start(
                            rw(re6[65 + j : 66 + j, :]),
                            rw(y2_rm[g : g + 1, j * 900 : (j + 1) * 900]),
                        )

                    wv_sb = SB.tile([1, EG], f32, tag="wv_sb", bufs=1, name=f"wv_sb{l}_{g}")
                    for hh in range(2):
                        cs = slice(hh * 450, (hh + 1) * 450)
                        # edge MLP stage 1 (pre = e_in @ e_W1)
                        m1 = []
                        for fb in range(2):
                            fs = slice(fb * 128, (fb + 1) * 128)
                            pre = EP2.tile([128, 450], f32, tag="ep2")
                            nc.tensor.matmul(
                                pre[:], rx(s70[:, fs]), rx(re6[:, cs]),
                                start=True, stop=True,
                            )
                            m1t = SB.tile([128, 450], f32, tag=f"m1_{fb}", bufs=4)
                            nc.scalar.activation(
                                rw(m1t[:]), pre[:], AF.Silu,
                                bias=t["eb1"][:, fb : fb + 1],
                            )
                            m1.append(m1t)
                        # edge MLP stage 2 (m = silu(m1 @ e_W2 + b2))
                        mg = []
                        for fb in range(2):
                            fs = slice(fb * 128, (fb + 1) * 128)
                            pre2 = EP2.tile([128, 450], f32, tag="ep2")
                            for kc in range(2):
                                nc.tensor.matmul(
                                    pre2[:], rx(t["ew2"][kc][:, fs]),
                                    rx(m1[kc][:]),
                                    start=(kc == 0), stop=(kc == 1),
                                )
                            mt = SB.tile([128, 450], f32, tag=f"m_{fb}", bufs=4)
                            nc.scalar.activation(
                                rw(mt[:]), pre2[:], AF.Silu,
                                bias=t["eb2"][:, fb : fb + 1],
                            )
                            mg.append(mt)
                        # coord gate hidden
                        mw = []
                        for fb in range(2):
                            fs = slice(fb * 128, (fb + 1) * 128)
                            prew = EP2.tile([128, 450], f32, tag="ep2")
                            for kc in range(2):
                                nc.tensor.matmul(
                                    prew[:], rx(t["cw1"][kc][:, fs]),
                                    rx(mg[kc][:]),
                                    start=(kc == 0), stop=(kc == 1),
                                )
                            mwt = SB.tile([128, 450], f32, tag=f"mw_{fb}")
                            nc.scalar.activation(
                                rw(mwt[:]), prew[:], AF.Silu,
                                bias=t["cb1"][:, fb : fb + 1],
                            )
                            mw.append(mwt)
                        # w = mw @ c_W2 -> [1, 450] (one psum bank per half)
                        wv = WP.tile([1, 450], f32, tag="wp", name=f"wv{l}_{g}_{hh}")
                        for kc in range(2):
                            nc.tensor.matmul(
                                wv[:], rx(t["cw2"][kc][:]), rx(mw[kc][:]),
                                start=(kc == 0), stop=(kc == 1),
                            )
                        nc.vector.tensor_copy(wv_sb[:, cs], wv[:])
                        # agg[f, a] += over this half's 15 a-rows
                        for fb in range(2):
                            with nc.allow_low_precision("f32r agg feeds f32r matmul"):
                                nc.vector.tensor_reduce(
                                    rw(agg[fb][:, g * PAD + hh * 15 : g * PAD + (hh + 1) * 15]),
                                    mg[fb][:].rearrange("p (a b) -> p a b", b=PAD),
                                    axis=AX.X,
                                    op=ALU.add,
                                )
                    # DVE cannot shift partitions: wv_sb staged on partition
                    # 0, DMA moves it to w_all row g.
                    nc.sync.dma_start(w_all[g : g + 1, :], wv_sb[:])

                # coord update: coordS += (sum_b cd*w) / 30
                for d in range(3):
                    trans = SB.tile([G, EG], f32, tag="trans", bufs=1, name=f"trans{l}_{d}")
                    nc.vector.tensor_mul(trans[:], cd[d][:], w_all[:])
                    ssum = SB.tile([G, PAD], f32, tag="ssum", bufs=1, name=f"ssum{l}_{d}")
                    nc.vector.tensor_reduce(
                        ssum[:],
                        trans[:].rearrange("g (a b) -> g a b", b=PAD),
                        axis=AX.X,
                        op=ALU.add,
                    )
                    nc.vector.scalar_tensor_tensor(
                        coordS[:, d * PAD : (d + 1) * PAD],
                        ssum[:],
                        1.0 / PAD,
                        coordS[:, d * PAD : (d + 1) * PAD],
                        op0=ALU.mult,
                        op1=ALU.add,
                    )

                # node update
                s1 = []
                for fb in range(2):
                    hn = EP.tile([128, NL], f32, tag="ep")
                    for kc in range(2):
                        nc.tensor.matmul(
                            hn[:],
                            rx(t["nw1h"][kc][:, fb * 128 : (fb + 1) * 128]),
                            rx(h_cur[kc][:]),
                            start=(kc == 0),
                            stop=False,
                        )
                    for kc in range(2):
                        nc.tensor.matmul(
                            hn[:],
                            rx(t["nw1a"][kc][:, fb * 128 : (fb + 1) * 128]),
                            rx(agg[kc][:]),
                            start=False,
                            stop=(kc == 1),
                        )
                    s1t = SB.tile([128, NL], f32, tag=f"s1_{fb}", bufs=1)
                    nc.scalar.activation(
                        rw(s1t[:]), hn[:], AF.Silu, bias=t["nb1"][:, fb : fb + 1]
                    )
                    s1.append(s1t)
                h_new = []
                for fb in range(2):
                    hp2 = EP.tile([128, NL], f32, tag="ep")
                    for kc in range(2):
                        nc.tensor.matmul(
                            hp2[:],
                            rx(t["nw2"][kc][:, fb * 128 : (fb + 1) * 128]),
                            rx(s1[kc][:]),
                            start=(kc == 0),
                            stop=(kc == 1),
                        )
                    ht = SB.tile([128, NL], f32, tag=f"h{fb}")
                    nc.scalar.activation(
                        rw(ht[:]), hp2[:], AF.Identity, bias=t["nb2"][:, fb : fb + 1]
                    )
                    h_new.append(ht)
                h_cur = h_new

            # ---------------- output heads ----------------
            # coord first so its (scattered) store overlaps the h_out head
            coord3 = coord_d.ap().rearrange("(g a) d -> g d a", a=PAD)
            for d, e in zip(range(3), (nc.sync, nc.scalar, nc.gpsimd)):
                e.dma_start(
                    coord3[:, d, :], coordS[:, d * PAD : (d + 1) * PAD]
                )
            r1p = EP.tile([128, NL], f32, tag="ep")
            for kc in range(2):
                nc.tensor.matmul(
                    r1p[:], rx(rn1_t[kc][:]), rx(h_cur[kc][:]),
                    start=(kc == 0), stop=(kc == 1),
                )
            r1s = SB.tile([128, NL], f32, tag="r1s")
            nc.scalar.activation(r1s[:], r1p[:], AF.Silu, bias=rnb1_t[:, 0:1])
            # node-major blocks: contiguous h_out rows, bias via ones row
            for c in range(4):
                nbl = slice(c * 120, (c + 1) * 120)
                hop = EP.tile([120, 5], f32, tag="ep", name=f"hop{c}")
                nc.tensor.matmul(
                    hop[:], r1s[:, nbl], rn2_t[:],
                    start=True, stop=False,
                )
                nc.tensor.matmul(
                    hop[:], ones128_t[:, 0:120], rnb2r_t[:],
                    start=False, stop=True,
                )
                hos = SB.tile([120, 5], f32, tag="hos", bufs=2, name=f"hos{c}")
                nc.vector.tensor_copy(hos[:], hop[:])
                nc.sync.dma_start(hout_d.ap()[nbl, :], hos[:])

    import concourse.mybir as mybir  # noqa: F811

    if legalize:
        _legalize_waits(nc, mybir)
    return nc


def _prep_weights(params):
    """Host-side weight massaging -> dict of named f32 arrays."""
    def A(x):
        return np.ascontiguousarray(np.asarray(x), dtype=np.float32)

    def col2(b):  # [256] -> [128, 2], column fb = features fb*128..
        return np.ascontiguousarray(A(b).reshape(2, 128).T)

    p = {}
    p["embw"] = A(params["emb_W"])
    p["embb"] = col2(params["emb_b"])
    rp = params["re_pos"]
    p["rp1"] = A(rp["W1"])
    p["rpb1"] = A(rp["b1"]).reshape(15, 1)
    p["rp2"] = A(rp["W2"])
    p["rpb2"] = A(rp["b2"]).reshape(3, 1)
    re = params["re_edge"]
    p["re1"] = np.ascontiguousarray(
        A(re["W1"]).reshape(PAD, ZIN, 60).transpose(1, 0, 2).reshape(ZIN, PAD * 60)
    )
    p["reb1"] = A(re["b1"]).reshape(60, 1)
    W2p = A(re["W2"]).reshape(60, 900, 5).transpose(0, 2, 1).reshape(60, 4500)
    b2p = A(re["b2"]).reshape(900, 5).T.reshape(1, 4500)
    p["re2b"] = np.ascontiguousarray(np.concatenate([W2p, b2p], axis=0))
    p["re2c"] = np.ascontiguousarray(
        np.concatenate([A(re["W2"]), A(re["b2"]).reshape(1, 4500)], axis=0)
    )
    p["ones16"] = np.ones((1, 128), dtype=np.float32)
    an = params["atom_num"]
    p["an1"] = np.ascontiguousarray(
        A(an["W1"]).reshape(PAD, ZIN, 60).transpose(1, 0, 2).reshape(ZIN, PAD * 60)
    )
    p["anb1"] = A(an["b1"]).reshape(60, 1)
    p["an2"] = A(an["W2"])
    p["anb2"] = A(an["b2"]).reshape(PAD, 1)
    rn = params["re_nodes"]
    p["idm"] = np.eye(128, dtype=np.float32)
    p["rn1"] = A(rn["W1"])
    p["rnb1"] = A(rn["b1"]).reshape(128, 1)
    p["rn2"] = A(rn["W2"])
    p["rnb2"] = A(rn["b2"]).reshape(5, 1)
    p["rnb2r"] = A(rn["b2"]).reshape(1, 5)

    # static one-hot broadcast/tile pattern
    bc = np.zeros((64, EG), dtype=np.float32)
    for a in range(PAD):
        bc[a, a * PAD : (a + 1) * PAD] = 1.0
    for b in range(PAD):
        bc[32 + b, b::PAD] = 1.0
    p["BC"] = bc

    blocks = []
    for lp in params["layers"]:
        eW1 = A(lp["e_W1"])
        nW1 = A(lp["n_W1"])
        for arr in (
            eW1[0:256],
            eW1[256:512],
            A(lp["e_W2"]),
            A(lp["c_W1"]),
            nW1[0:256] + nW1[512:768],
            nW1[256:512],
            A(lp["n_W2"]),
        ):
            blocks.append(arr[0:128, :])
            blocks.append(arr[128:256, :])
    blocks.append(A(rn["W1"])[0:128, :])
    blocks.append(A(rn["W1"])[128:256, :])
    for lp in params["layers"]:
        cw2 = A(lp["c_W2"]).reshape(HID, 1)
        blocks.append(cw2[0:128, :])
        blocks.append(cw2[128:256, :])
    for lp in params["layers"]:
        for b in (lp["e_b1"], lp["e_b2"], lp["c_b1"], lp["n_b1"], lp["n_b2"]):
            blocks.append(col2(b))
    p["big0"] = np.ascontiguousarray(np.concatenate(blocks, axis=1))
    assert p["big0"].shape == (128, 7448), p["big0"].shape

    for l, lp in enumerate(params["layers"]):
        eW1 = A(lp["e_W1"])  # [518, 256]
        p[f"ehr{l}"] = np.ascontiguousarray(eW1[0:256])
        p[f"ehc{l}"] = np.ascontiguousarray(eW1[256:512])
        wz = np.zeros((10, HID), dtype=np.float32)
        wz[4] = eW1[512]
        wz[5:10] = eW1[513:518]
        p[f"wz{l}"] = wz
        p[f"eb1_{l}"] = col2(lp["e_b1"])
        p[f"ew2_{l}"] = A(lp["e_W2"])
        p[f"eb2_{l}"] = col2(lp["e_b2"])
        p[f"cw1_{l}"] = A(lp["c_W1"])
        p[f"cb1_{l}"] = col2(lp["c_b1"])
        p[f"cw2_{l}"] = A(lp["c_W2"]).reshape(HID, 1)
        nW1 = A(lp["n_W1"])  # [768, 256]
        p[f"nw1h_{l}"] = np.ascontiguousarray(nW1[0:256] + nW1[512:768])
        p[f"nw1a_{l}"] = np.ascontiguousarray(nW1[256:512])
        p[f"nb1_{l}"] = col2(lp["n_b1"])
        p[f"nw2_{l}"] = A(lp["n_W2"])
        p[f"nb2_{l}"] = col2(lp["n_b2"])
    return p


def _expected_edges():
    i = np.arange(PAD)
    row = np.repeat(i, PAD)
    col = np.tile(i, PAD)
    offs = (np.arange(B) * PAD)[:, None]
    return np.stack(
        [
            (row[None, :] + offs).reshape(-1),
            (col[None, :] + offs).reshape(-1),
        ]
    ).astype(np.int64)


def _get_nc():
    if "nc" not in _CACHE:
        _CACHE["nc"] = _build_program()
    return _CACHE["nc"]


def _install_ntff_hook():
    """antenv.axon_hooks is absent in this image; shim it and register the
    ctypes NTFF profiling hook from trn_agent_boot so trace=True works."""
    import types
    import importlib

    try:
        importlib.import_module("antenv.axon_hooks")
        return
    except ImportError:
        pass
    try:
        from trn_agent_boot.trn_boot import _ntff_profile_via_ctypes
    except ImportError:
        return
    import antenv

    mod = types.ModuleType("antenv.axon_hooks")
    mod._hook = _ntff_profile_via_ctypes("/opt/axon/libaxon_pjrt.so")

    def set_axon_ntff_profile_hook(h):
        mod._hook = h

    def get_axon_ntff_profile_hook():
        return mod._hook

    mod.set_axon_ntff_profile_hook = set_axon_ntff_profile_hook
    mod.get_axon_ntff_profile_hook = get_axon_ntff_profile_hook
    sys.modules["antenv.axon_hooks"] = mod
    antenv.axon_hooks = mod


def kernel(z, label, edges, n_nodes, params, _trace=False):
    from concourse.bass_utils import run_bass_kernel_spmd

    z = np.asarray(z, dtype=np.float32)
    label = np.asarray(label, dtype=np.float32)
    edges_np = np.asarray(edges)
    assert int(n_nodes) == PAD, f"n_nodes must be {PAD}"
    if not np.array_equal(edges_np.astype(np.int64), _expected_edges()):
        raise ValueError("edges do not match the full per-graph grid layout")

    wmap = _prep_weights(params)
    nc = _get_nc()

    in_maps = []
    for c in range(NCORES):
        ns = slice(c * NL, (c + 1) * NL)
        m = dict(wmap)
        m["z"] = np.ascontiguousarray(z[ns])
        m["lab"] = np.ascontiguousarray(label[ns])
        in_maps.append(m)

    kw = {}
    if _trace:
        _install_ntff_hook()
        kw = dict(trace=True, trace_cores=[0])
    res = run_bass_kernel_spmd(nc, in_maps, core_ids=list(range(NCORES)), **kw)

    h_out = np.concatenate([res.results[c]["h_out"] for c in range(NCORES)], 0)
    coord = np.concatenate(
        [res.results[c]["coord_out"] for c in range(NCORES)], 0
    )
    ea = np.concatenate([res.results[c]["ea_out"] for c in range(NCORES)], 0)
    an = np.concatenate([res.results[c]["an_out"] for c in range(NCORES)], 0)
    if _trace:
        return (h_out, coord, ea, an), res
    return h_out, coord, ea, an


# revision 69
# speedup vs baseline: 1.0037x; 1.0037x over previous
"""Trainium2 Bass kernel for nn_Decoder_78176994721983 (EGNN-style decoder).

Data-parallel over graphs: 128 graphs x 30 padded nodes, sharded as 16
graphs per NeuronCore across 8 cores. All segment ops are device-local
(edges form a full 30x30 grid inside each graph, so gathers/scatters
become structured broadcast matmuls and grouped free-dim reductions).

Key algorithmic restructuring vs the reference:
  - e_in = [h[row], h[col], radial, edge_attr] @ e_W1 is decomposed into
    node-level pre-products P = h @ e_W1[:256], Q = h @ e_W1[256:512]
    plus an edge-level K=70 matmul whose stationary operand stacks
    [P_g (30 rows), 0, Q_g (30 rows at partition 32), 0, W1_radial,
    W1_edge_attr] and whose moving operand is a static one-hot
    broadcast/tile pattern with radial and edge_attr rows appended.
  - segment_sum over rows = grouped reduction over the inner 30 (col)
    axis of [*, 30a, 30b] views; cnt == 30 exactly (full grid).
  - coord update folds 1/30 into a scalar_tensor_tensor.
"""

import os
import sys

if "/opt/trn_rl_repo" not in sys.path:
    sys.path.insert(0, "/opt/trn_rl_repo")

import numpy as np

B, PAD, LAT, HID = 128, 30, 64, 256
ZIN = LAT + 7  # 71
NCORES = 8
G = B // NCORES            # graphs per core: 16
NL = G * PAD               # nodes per core: 480
EL = G * PAD * PAD         # edges per core: 14400
EG = PAD * PAD             # edges per graph: 900
N_LAYERS = 2

# matmul dtype policy: relaxed fp32 (single-pass PE mode, 4x faster at
# free-dim >= 256) for the big edge matmuls; plain fp32 otherwise.
RELAXED = os.environ.get("KERNEL_F32R", "1") != "0"

_CACHE = {}


# --------------------------------------------------------------------------
# walrus in this container accepts at most ONE sync-wait per instruction;
# Tile emits several. Split extras onto same-engine no-ops just before the
# instruction (same sequencer stream => identical blocking semantics).
def _legalize_waits(nc, mybir):
    n_split = 0
    for f in nc.m.functions:
        for blk in f.blocks:
            insts = list(blk.instructions)
            out = []
            changed = False
            for inst in insts:
                si = inst.sync_info
                if si is not None:
                    waits = list(si.on_wait)
                    if len(waits) > 1:
                        changed = True
                        n_split += 1
                        for j, w in enumerate(waits[:-1]):
                            out.append(
                                mybir.InstNoOp(
                                    name=f"{inst.name}-w{j}",
                                    sync_info=mybir.SyncInfo(
                                        on_wait=[w], on_update=[]
                                    ),
                                    bass_nofuse=True,
                                    engine=inst.engine,
                                )
                            )
                        si.on_wait = waits[-1:]
                out.append(inst)
            if changed:
                blk.instructions = out
    return n_split


def _build_program(legalize=True):
    import concourse.bass as bass
    import concourse.mybir as mybir
    import concourse.tile as tile

    f32 = mybir.dt.float32
    f32r = mybir.dt.float32r
    AF = mybir.ActivationFunctionType
    ALU = mybir.AluOpType
    AX = mybir.AxisListType

    def rx(ap):
        return ap.bitcast(f32r) if RELAXED else ap

    # The BIR verifier requires every producer of an fp32r-matmul operand
    # to emit fp32r-rounded output: write those tensors through fp32r-
    # bitcast views (engines round on the output cast). Non-matmul readers
    # keep plain f32 views of the same bits.
    rw = rx

    nc = bass.Bass()

    def din(name, shape):
        return nc.dram_tensor(name, list(shape), f32, kind="ExternalInput")

    def dout(name, shape):
        return nc.dram_tensor(name, list(shape), f32, kind="ExternalOutput")

    z_d = din("z", [NL, LAT])
    lab_d = din("lab", [NL, 7])
    bc_d = din("BC", [64, EG])
    embw_d = din("embw", [ZIN, HID])
    embb_d = din("embb", [128, 2])
    rp1_d = din("rp1", [ZIN, 15])
    rpb1_d = din("rpb1", [15, 1])
    rp2_d = din("rp2", [15, 3])
    rpb2_d = din("rpb2", [3, 1])
    re1_d = din("re1", [ZIN, 30 * 60])
    reb1_d = din("reb1", [60, 1])
    re2b_d = din("re2b", [61, 4500])
    re2c_d = din("re2c", [61, 4500])
    ones16_d = din("ones16", [1, 128])
    rnb2r_d = din("rnb2r", [1, 5])
    an1_d = din("an1", [ZIN, 30 * 60])
    anb1_d = din("anb1", [60, 1])
    an2_d = din("an2", [60, PAD])
    anb2_d = din("anb2", [PAD, 1])
    idm_d = din("idm", [128, 128])
    rn1_d = din("rn1", [HID, 128])
    big0_d = din("big0", [128, 7448])
    rnb1_d = din("rnb1", [128, 1])
    rn2_d = din("rn2", [128, 5])
    rnb2_d = din("rnb2", [5, 1])

    L = []
    for l in range(N_LAYERS):
        L.append(
            {
                "ehr": din(f"ehr{l}", [HID, HID]),
                "ehc": din(f"ehc{l}", [HID, HID]),
                "wz": din(f"wz{l}", [10, HID]),
                "eb1": din(f"eb1_{l}", [128, 2]),
                "ew2": din(f"ew2_{l}", [HID, HID]),
                "eb2": din(f"eb2_{l}", [128, 2]),
                "cw1": din(f"cw1_{l}", [HID, HID]),
                "cb1": din(f"cb1_{l}", [128, 2]),
                "cw2": din(f"cw2_{l}", [HID, 1]),
                "nw1h": din(f"nw1h_{l}", [HID, HID]),
                "nw1a": din(f"nw1a_{l}", [HID, HID]),
                "nb1": din(f"nb1_{l}", [128, 2]),
                "nw2": din(f"nw2_{l}", [HID, HID]),
                "nb2": din(f"nb2_{l}", [128, 2]),
            }
        )

    hout_d = dout("h_out", [NL, 5])
    coord_d = dout("coord_out", [NL, 3])
    ea_d = dout("ea_out", [EL, 5])
    an_d = dout("an_out", [G, PAD])

    with tile.TileContext(nc) as tc:
        with (
            tc.tile_pool(name="wb", bufs=1) as W,
            tc.tile_pool(name="sb", bufs=2) as SB,
            tc.tile_pool(name="ep", bufs=2, space="PSUM") as EP,
            tc.tile_pool(name="ep2", bufs=5, space="PSUM") as EP2,
            tc.tile_pool(name="wp", bufs=1, space="PSUM") as WP,
        ):
            # ---------------- static loads ----------------
            # Order matters: DMA queues are FIFO, so the tiny inputs that
            # gate the whole dependency tree (z/lab -> z_t transposes ->
            # heads/h/coord) go first, then the weights by first use,
            # with the layer-1 block last.
            idm_t = W.tile([128, 128], f32, tag="idm")
            nc.sync.dma_start(idm_t[:], idm_d.ap())
            z_t = W.tile([ZIN, NL], f32, tag="z_t")
            zls = []
            for c in range(4):
                nsl = slice(c * 120, (c + 1) * 120)
                zl = SB.tile([120, ZIN], f32, tag="zl", name=f"zl{c}")
                nc.sync.dma_start(zl[:, 0:LAT], z_d.ap()[nsl, :])
                nc.scalar.dma_start(zl[:, LAT:ZIN], lab_d.ap()[nsl, :])
                zls.append(zl)
            for c in range(4):
                nsl = slice(c * 120, (c + 1) * 120)
                zps = EP.tile([ZIN, 120], f32, tag="ep", name=f"zps{c}")
                nc.tensor.transpose(zps[:], zls[c][:], idm_t[0:120, 0:120])
                nc.vector.tensor_copy(z_t[:, nsl], zps[:])
            # re2b first: it gates y2_rm -> re6 edge_attr rows -> the very
            # first pre matmul of layer 0 (longest startup pole).
            re2b_t = W.tile([61, 4500], f32, tag="re2b")
            for ci, e in zip(range(3), (nc.scalar, nc.sync, nc.scalar)):
                csl = slice(ci * 1500, (ci + 1) * 1500)
                e.dma_start(rw(re2b_t[:, csl]), rw(re2b_d.ap()[:, csl]))
            big0_t = W.tile([128, 7448], f32, tag="big0")
            nc.sync.dma_start(rw(big0_t[:, 0:1024]), rw(big0_d.ap()[:, 0:1024]))
            nc.scalar.dma_start(
                rw(big0_t[:, 7168:7448]), rw(big0_d.ap()[:, 7168:7448])
            )
            nc.sync.dma_start(
                rw(big0_t[:, 1024:1536]), rw(big0_d.ap()[:, 1024:1536])
            )
            nc.scalar.dma_start(
                rw(big0_t[:, 1536:2048]), rw(big0_d.ap()[:, 1536:2048])
            )
            nc.sync.dma_start(
                rw(big0_t[:, 2048:3584]), rw(big0_d.ap()[:, 2048:3584])
            )
            # zg chunks: zg^T rows [a*71 .. a*71+71) for graph g are just
            # z_t columns g*30+a -> strided slices of z_t, no extra tile.
            zg3 = z_t[:].rearrange("j (g a) -> j a g", a=PAD)
            # Static one-hot broadcast/tile pattern, shared by all graphs.
            # The per-graph radial/edge_attr rows live in a separate
            # [70, 900] tile (rows 64..69) so the pre matmul is
            # K=64 (static) + K=6 (per graph) accumulating in PSUM.
            re6s = []
            for i in range(2):
                r = W.tile([70, EG], f32, tag=f"re6_{i}", name=f"re6s{i}")
                if i == 0:
                    # graph 0's tile gates the first pre matmul: fast
                    # split load on both HWDGE queues
                    nc.sync.dma_start(rw(r[0:32, :]), rw(bc_d.ap()[0:32, :]))
                    nc.scalar.dma_start(rw(r[32:64, :]), rw(bc_d.ap()[32:64, :]))
                else:
                    # off the critical path (first used at graph 1)
                    nc.gpsimd.dma_start(rw(r[0:64, :]), rw(bc_d.ap()))
                re6s.append(r)

            _eng_rot = [nc.sync, nc.scalar, nc.gpsimd]
            _eng_i = [0]

            def _eng():
                e = _eng_rot[_eng_i[0] % len(_eng_rot)]
                _eng_i[0] += 1
                return e

            def wload(d, shape, tag):
                t = W.tile(list(shape), f32, tag=tag)
                _eng().dma_start(t[:], d.ap())
                return t

            embw_t = wload(embw_d, [ZIN, HID], "embw")
            embb_t = wload(embb_d, [128, 2], "embb")
            rp1_t = wload(rp1_d, [ZIN, 15], "rp1")
            rpb1_t = wload(rpb1_d, [15, 1], "rpb1")
            rp2_t = wload(rp2_d, [15, 3], "rp2")
            rpb2_t = wload(rpb2_d, [3, 1], "rpb2")
            re1_t = wload(re1_d, [ZIN, 1800], "re1")
            reb1_t = wload(reb1_d, [60, 1], "reb1")
            re2c_t = W.tile([61, 4500], f32, tag="re2c")
            nc.gpsimd.dma_start(rw(re2c_t[:]), rw(re2c_d.ap()))
            an1_t = wload(an1_d, [ZIN, 1800], "an1")
            anb1_t = wload(anb1_d, [60, 1], "anb1")
            an2_t = wload(an2_d, [60, PAD], "an2")
            anb2_t = wload(anb2_d, [PAD, 1], "anb2")
            rnb1_t = wload(rnb1_d, [128, 1], "rnb1")
            rn2_t = W.tile([128, 5], f32, tag="rn2")
            nc.sync.dma_start(rn2_t[:], rn2_d.ap())
            ones128_t = W.tile([1, 128], f32, tag="ones128")
            nc.scalar.dma_start(ones128_t[:], ones16_d.ap())
            rnb2r_t = W.tile([1, 5], f32, tag="rnb2r")
            nc.scalar.dma_start(rnb2r_t[:], rnb2r_d.ap())
            rn1_t = None  # assigned after the packed load below

            # all [128, x] layer weights packed host-side into one tensor
            # -> a single large contiguous DMA instead of ~32 small ones
            # deferred layer-1 weights (consumed ~halfway through)
            nc.sync.dma_start(
                rw(big0_t[:, 3584:7168]), rw(big0_d.ap()[:, 3584:7168])
            )
            _off = [0]

            def _blk(cols):
                o = _off[0]
                _off[0] += cols
                return big0_t[:, o : o + cols]

            LT = []
            for l in range(N_LAYERS):
                d = L[l]
                t = {}
                for nm in ("ehr", "ehc", "ew2", "cw1", "nw1h", "nw1a", "nw2"):
                    t[nm] = [_blk(HID), _blk(HID)]
                t["wz"] = d["wz"]  # stays in DRAM; DMA'd per graph
                LT.append(t)
            rn1_p = [_blk(128), _blk(128)]
            for l in range(N_LAYERS):
                LT[l]["cw2"] = [_blk(1), _blk(1)]
            for l in range(N_LAYERS):
                for nm in ("eb1", "eb2", "cb1", "nb1", "nb2"):
                    LT[l][nm] = _blk(2)
            rn1_t = rn1_p

            # ---------------- graph-level heads ----------------
            # re_edge hidden: y1 = silu(zg @ W1 + b1), K accumulated in
            # 30 chunks of 71 (one per node slot a).
            y1p = EP.tile([60, G], f32, tag="ep")
            for a in range(PAD):
                nc.tensor.matmul(
                    y1p[:],
                    re1_t[:, a * 60 : (a + 1) * 60],
                    zg3[:, a, :],
                    start=(a == 0),
                    stop=(a == PAD - 1),
                )
            # [y1; ones] so the next matmul folds the bias in
            y1s = SB.tile([61, G], f32, tag="y1s")
            nc.scalar.activation(
                rw(y1s[0:60, :]), y1p[:], AF.Silu, bias=reb1_t[:, 0:1]
            )
            nc.sync.dma_start(rw(y1s[60:61, :]), rw(ones16_d.ap()[:, 0:G]))
            # atom_num hidden
            ay1p = EP.tile([60, G], f32, tag="ep")
            for a in range(PAD):
                nc.tensor.matmul(
                    ay1p[:],
                    an1_t[:, a * 60 : (a + 1) * 60],
                    zg3[:, a, :],
                    start=(a == 0),
                    stop=(a == PAD - 1),
                )
            ay1s = SB.tile([60, G], f32, tag="ay1s")
            nc.scalar.activation(ay1s[:], ay1p[:], AF.Silu, bias=anb1_t[:, 0:1])
            # re_edge out, graph-major with host-permuted columns:
            # y2_rm[g, j*900+k] = edge_attr[g*900+k, j]; bias folded via
            # the ones row of y1s.
            y2_rm = SB.tile([G, 4500], f32, tag="y2_rm", bufs=1)
            for fb in range(9):
                fsl = slice(fb * 500, (fb + 1) * 500)
                y2p = EP.tile([G, 500], f32, tag="ep", name=f"y2p{fb}")
                nc.tensor.matmul(
                    y2p[:], rx(y1s[:]), rx(re2b_t[:, fsl]),
                    start=True, stop=True,
                )
                nc.vector.tensor_copy(y2_rm[:, fsl], y2p[:])
            # edge_attr output: second head pass with unpermuted columns
            # so the DRAM write is fully contiguous (16 descriptors/block)
            ea2d = ea_d.ap().rearrange("e j -> (e j)").rearrange(
                "(g f) -> g f", g=G
            )
            for fb in range(9):
                fsl = slice(fb * 500, (fb + 1) * 500)
                eap = EP.tile([G, 500], f32, tag="ep", name=f"eap{fb}")
                nc.tensor.matmul(
                    eap[:], rx(y1s[:]), rx(re2c_t[:, fsl]),
                    start=True, stop=True,
                )
                eas = SB.tile([G, 500], f32, tag="eas", bufs=2, name=f"eas{fb}")
                nc.vector.tensor_copy(eas[:], eap[:])
                nc.sync.dma_start(ea2d[:, fsl], eas[:])
            # atom_num out
            ayp = EP.tile([PAD, G], f32, tag="ep")
            nc.tensor.matmul(ayp[:], an2_t[:], ay1s[:], start=True, stop=True)
            ay_sb = SB.tile([PAD, G], f32, tag="ay_sb")
            nc.scalar.activation(
                ay_sb[:], ayp[:], AF.Identity, bias=anb2_t[:, 0:1]
            )
            nc.sync.dma_start(
                an_d.ap().rearrange("g k -> k g"), ay_sb[:]
            )


            # ---------------- h / coord init ----------------
            h_cur = []
            for fb in range(2):
                hp = EP.tile([128, NL], f32, tag="ep")
                nc.tensor.matmul(
                    hp[:], embw_t[:, fb * 128 : (fb + 1) * 128], z_t[:],
                    start=True, stop=True,
                )
                ht = SB.tile([128, NL], f32, tag=f"h{fb}")
                nc.scalar.activation(
                    rw(ht[:]), hp[:], AF.Identity, bias=embb_t[:, fb : fb + 1]
                )
                h_cur.append(ht)
            cp1 = EP.tile([15, NL], f32, tag="ep")
            nc.tensor.matmul(cp1[:], rp1_t[:], z_t[:], start=True, stop=True)
            c1s = SB.tile([15, NL], f32, tag="c1s")
            nc.scalar.activation(c1s[:], cp1[:], AF.Silu, bias=rpb1_t[:, 0:1])
            cp2 = EP.tile([3, NL], f32, tag="ep")
            nc.tensor.matmul(cp2[:], rp2_t[:], c1s[:], start=True, stop=True)
            coordT = SB.tile([3, NL], f32, tag="coordT")
            nc.scalar.activation(
                coordT[:], cp2[:], AF.Identity, bias=rpb2_t[:, 0:1]
            )
            # coordS[g, d*30+a] = coord[g*30+a, d]
            coordS = SB.tile([G, 3 * PAD], f32, tag="coordS", bufs=1)
            for d in range(3):
                # shapes differ but iteration orders match: (g, a) vs g*30+a
                nc.sync.dma_start(
                    coordS[:, d * PAD : (d + 1) * PAD],
                    coordT[d : d + 1, :],
                )

            # ---------------- message-passing layers ----------------
            for l in range(N_LAYERS):
                t = LT[l]
                # coordinate differences, stacked graphs on partitions
                cd = []
                for d in range(3):
                    cdt = SB.tile([G, EG], f32, tag=f"cd{d}", bufs=1)
                    ca = coordS[:, d * PAD : (d + 1) * PAD]
                    nc.vector.tensor_sub(
                        cdt[:].rearrange("g (a b) -> g a b", b=PAD),
                        ca.broadcast_to([G, PAD, PAD]),
                        ca[:, None, :].broadcast_to([G, PAD, PAD]),
                    )
                    cd.append(cdt)
                radial = SB.tile([G, EG], f32, tag="radial", bufs=1)
                sqt = SB.tile([G, EG], f32, tag="trans", bufs=1)
                nc.vector.tensor_mul(radial[:], cd[0][:], cd[0][:])
                nc.vector.tensor_mul(sqt[:], cd[1][:], cd[1][:])
                nc.vector.tensor_add(radial[:], radial[:], sqt[:])
                nc.vector.tensor_mul(sqt[:], cd[2][:], cd[2][:])
                nc.vector.tensor_add(radial[:], radial[:], sqt[:])
                agg = [
                    SB.tile([128, NL], f32, tag=f"agg{fb}", bufs=1, name=f"agg{l}_{fb}")
                    for fb in range(2)
                ]
                w_all = SB.tile([G, EG], f32, tag="w_all", bufs=1)

                for g in range(G):
                    ns = slice(g * PAD, (g + 1) * PAD)
                    # P and Q node-level pre-products, both M=30 at (0,0)
                    # (the fp32r ISA check rejects col-offset tile_position,
                    # so Q reaches s70 rows 32..61 via a staging DMA).
                    psP = EP.tile([30, HID], f32, tag="ep", name=f"psP{l}_{g}")
                    for kc in range(2):
                        nc.tensor.matmul(
                            psP[:],
                            rx(h_cur[kc][:, ns]),
                            rx(t["ehr"][kc][:]),
                            start=(kc == 0),
                            stop=(kc == 1),
                        )
                    psQ = EP.tile([30, HID], f32, tag="ep", name=f"psQ{l}_{g}")
                    for kc in range(2):
                        nc.tensor.matmul(
                            psQ[:],
                            rx(h_cur[kc][:, ns]),
                            rx(t["ehc"][kc][:]),
                            start=(kc == 0),
                            stop=(kc == 1),
                        )
                    s70 = SB.tile([70, HID], f32, tag="s70", bufs=4)
                    nc.sync.dma_start(rw(s70[30:32, :]), rw(t["wz"].ap()[0:2, :]))
                    nc.sync.dma_start(rw(s70[62:70, :]), rw(t["wz"].ap()[2:10, :]))
                    nc.vector.tensor_copy(rw(s70[0:30, :]), psP[:])
                    qst = SB.tile([30, HID], f32, tag="qst", bufs=3, name=f"qst{l}_{g}")
                    nc.vector.tensor_copy(rw(qst[:]), psQ[:])
                    nc.sync.dma_start(rw(s70[32:62, :]), rw(qst[:]))
                    # per-graph radial + edge_attr rows at partitions
                    # 64..69 of the static ping-pong tile (BC rows 0..63
                    # were filled once at setup -> single K=70 pre matmul)
                    re6 = re6s[g % 2]
                    nc.sync.dma_start(rw(re6[64:65, :]), rw(radial[g : g + 1, :]))
                    for j in range(5):
                        nc.sync.dma_start(
                            rw(re6[65 + j : 66 + j, :]),
                            rw(y2_rm[g : g + 1, j * 900 : (j + 1) * 900]),
                        )

                    wv_sb = SB.tile([1, EG], f32, tag="wv_sb", bufs=1, name=f"wv_sb{l}_{g}")
                    for hh in range(2):
                        cs = slice(hh * 450, (hh + 1) * 450)
                        # edge MLP stage 1 (pre = e_in @ e_W1)
                        m1 = []
                        for fb in range(2):
                            fs = slice(fb * 128, (fb + 1) * 128)
                            pre = EP2.tile([128, 450], f32, tag="ep2")
                            nc.tensor.matmul(
                                pre[:], rx(s70[:, fs]), rx(re6[:, cs]),
                                start=True, stop=True,
                            )
                            m1t = SB.tile([128, 450], f32, tag=f"m1_{fb}", bufs=4)
                            nc.scalar.activation(
                                rw(m1t[:]), pre[:], AF.Silu,
                                bias=t["eb1"][:, fb : fb + 1],
                            )
                            m1.append(m1t)
                        # edge MLP stage 2 (m = silu(m1 @ e_W2 + b2))
                        mg = []
                        for fb in range(2):
                            fs = slice(fb * 128, (fb + 1) * 128)
                            pre2 = EP2.tile([128, 450], f32, tag="ep2")
                            for kc in range(2):
                                nc.tensor.matmul(
                                    pre2[:], rx(t["ew2"][kc][:, fs]),
                                    rx(m1[kc][:]),
                                    start=(kc == 0), stop=(kc == 1),
                                )
                            mt = SB.tile([128, 450], f32, tag=f"m_{fb}", bufs=4)
                            nc.scalar.activation(
                                rw(mt[:]), pre2[:], AF.Silu,
                                bias=t["eb2"][:, fb : fb + 1],
                            )
                            mg.append(mt)
                        # coord gate hidden
                        mw = []
                        for fb in range(2):
                            fs = slice(fb * 128, (fb + 1) * 128)
                            prew = EP2.tile([128, 450], f32, tag="ep2")
                            for kc in range(2):
                                nc.tensor.matmul(
                                    prew[:], rx(t["cw1"][kc][:, fs]),
                                    rx(mg[kc][:]),
                                    start=(kc == 0), stop=(kc == 1),
                                )
                            mwt = SB.tile([128, 450], f32, tag=f"mw_{fb}")
                            nc.scalar.activation(
                                rw(mwt[:]), prew[:], AF.Silu,
                                bias=t["cb1"][:, fb : fb + 1],
                            )
                            mw.append(mwt)
                        # w = mw @ c_W2 -> [1, 450] (one psum bank per half)
                        wv = WP.tile([1, 450], f32, tag="wp", name=f"wv{l}_{g}_{hh}")
                        for kc in range(2):
                            nc.tensor.matmul(
                                wv[:], rx(t["cw2"][kc][:]), rx(mw[kc][:]),
                                start=(kc == 0), stop=(kc == 1),
                            )
                        nc.vector.tensor_copy(wv_sb[:, cs], wv[:])
                        # agg[f, a] += over this half's 15 a-rows
                        for fb in range(2):
                            with nc.allow_low_precision("f32r agg feeds f32r matmul"):
                                nc.vector.tensor_reduce(
                                    rw(agg[fb][:, g * PAD + hh * 15 : g * PAD + (hh + 1) * 15]),
                                    mg[fb][:].rearrange("p (a b) -> p a b", b=PAD),
                                    axis=AX.X,
                                    op=ALU.add,
                                )
                    # DVE cannot shift partitions: wv_sb staged on partition
                    # 0, DMA moves it to w_all row g.
                    nc.sync.dma_start(w_all[g : g + 1, :], wv_sb[:])

                # coord update: coordS += (sum_b cd*w) / 30
                for d in range(3):
                    trans = SB.tile([G, EG], f32, tag="trans", bufs=1, name=f"trans{l}_{d}")
                    nc.vector.tensor_mul(trans[:], cd[d][:], w_all[:])
                    ssum = SB.tile([G, PAD], f32, tag="ssum", bufs=1, name=f"ssum{l}_{d}")
                    nc.vector.tensor_reduce(
                        ssum[:],
                        trans[:].rearrange("g (a b) -> g a b", b=PAD),
                        axis=AX.X,
                        op=ALU.add,
                    )
                    nc.vector.scalar_tensor_tensor(
                        coordS[:, d * PAD : (d + 1) * PAD],
                        ssum[:],
                        1.0 / PAD,
                        coordS[:, d * PAD : (d + 1) * PAD],
                        op0=ALU.mult,
                        op1=ALU.add,
                    )

                # node update
                s1 = []
                for fb in range(2):
                    hn = EP.tile([128, NL], f32, tag="ep")
                    for kc in range(2):
                        nc.tensor.matmul(
                            hn[:],
                            rx(t["nw1h"][kc][:, fb * 128 : (fb + 1) * 128]),
                            rx(h_cur[kc][:]),
                            start=(kc == 0),
                            stop=False,
                        )
                    for kc in range(2):
                        nc.tensor.matmul(
                            hn[:],
                            rx(t["nw1a"][kc][:, fb * 128 : (fb + 1) * 128]),
                            rx(agg[kc][:]),
                            start=False,
                            stop=(kc == 1),
                        )
                    s1t = SB.tile([128, NL], f32, tag=f"s1_{fb}", bufs=1)
                    nc.scalar.activation(
                        rw(s1t[:]), hn[:], AF.Silu, bias=t["nb1"][:, fb : fb + 1]
                    )
                    s1.append(s1t)
                h_new = []
                for fb in range(2):
                    hp2 = EP.tile([128, NL], f32, tag="ep")
                    for kc in range(2):
                        nc.tensor.matmul(
                            hp2[:],
                            rx(t["nw2"][kc][:, fb * 128 : (fb + 1) * 128]),
                            rx(s1[kc][:]),
                            start=(kc == 0),
                            stop=(kc == 1),
                        )
                    ht = SB.tile([128, NL], f32, tag=f"h{fb}")
                    nc.scalar.activation(
                        rw(ht[:]), hp2[:], AF.Identity, bias=t["nb2"][:, fb : fb + 1]
                    )
                    h_new.append(ht)
                h_cur = h_new

            # ---------------- output heads ----------------
            # coord first so its (scattered) store overlaps the h_out head
            coord3 = coord_d.ap().rearrange("(g a) d -> g d a", a=PAD)
            for d, e in zip(range(3), (nc.sync, nc.scalar, nc.gpsimd)):
                e.dma_start(
                    coord3[:, d, :], coordS[:, d * PAD : (d + 1) * PAD]
                )
            r1p = EP.tile([128, NL], f32, tag="ep")
            for kc in range(2):
                nc.tensor.matmul(
                    r1p[:], rx(rn1_t[kc][:]), rx(h_cur[kc][:]),
                    start=(kc == 0), stop=(kc == 1),
                )
            r1s = SB.tile([128, NL], f32, tag="r1s")
            nc.scalar.activation(r1s[:], r1p[:], AF.Silu, bias=rnb1_t[:, 0:1])
            # node-major blocks: contiguous h_out rows, bias via ones row
            for c in range(4):
                nbl = slice(c * 120, (c + 1) * 120)
                hop = EP.tile([120, 5], f32, tag="ep", name=f"hop{c}")
                nc.tensor.matmul(
                    hop[:], r1s[:, nbl], rn2_t[:],
                    start=True, stop=False,
                )
                nc.tensor.matmul(
                    hop[:], ones128_t[:, 0:120], rnb2r_t[:],
                    start=False, stop=True,
                )
                hos = SB.tile([120, 5], f32, tag="hos", bufs=2, name=f"hos{c}")
                nc.vector.tensor_copy(hos[:], hop[:])
                nc.sync.dma_start(hout_d.ap()[nbl, :], hos[:])

    import concourse.mybir as mybir  # noqa: F811

    if legalize:
        _legalize_waits(nc, mybir)
    return nc


def _prep_weights(params):
    """Host-side weight massaging -> dict of named f32 arrays."""
    def A(x):
        return np.ascontiguousarray(np.asarray(x), dtype=np.float32)

    def col2(b):  # [256] -> [128, 2], column fb = features fb*128..
        return np.ascontiguousarray(A(b).reshape(2, 128).T)

    p = {}
    p["embw"] = A(params["emb_W"])
    p["embb"] = col2(params["emb_b"])
    rp = params["re_pos"]
    p["rp1"] = A(rp["W1"])
    p["rpb1"] = A(rp["b1"]).reshape(15, 1)
    p["rp2"] = A(rp["W2"])
    p["rpb2"] = A(rp["b2"]).reshape(3, 1)
    re = params["re_edge"]
    p["re1"] = np.ascontiguousarray(
        A(re["W1"]).reshape(PAD, ZIN, 60).transpose(1, 0, 2).reshape(ZIN, PAD * 60)
    )
    p["reb1"] = A(re["b1"]).reshape(60, 1)
    W2p = A(re["W2"]).reshape(60, 900, 5).transpose(0, 2, 1).reshape(60, 4500)
    b2p = A(re["b2"]).reshape(900, 5).T.reshape(1, 4500)
    p["re2b"] = np.ascontiguousarray(np.concatenate([W2p, b2p], axis=0))
    p["re2c"] = np.ascontiguousarray(
        np.concatenate([A(re["W2"]), A(re["b2"]).reshape(1, 4500)], axis=0)
    )
    p["ones16"] = np.ones((1, 128), dtype=np.float32)
    an = params["atom_num"]
    p["an1"] = np.ascontiguousarray(
        A(an["W1"]).reshape(PAD, ZIN, 60).transpose(1, 0, 2).reshape(ZIN, PAD * 60)
    )
    p["anb1"] = A(an["b1"]).reshape(60, 1)
    p["an2"] = A(an["W2"])
    p["anb2"] = A(an["b2"]).reshape(PAD, 1)
    rn = params["re_nodes"]
    p["idm"] = np.eye(128, dtype=np.float32)
    p["rn1"] = A(rn["W1"])
    p["rnb1"] = A(rn["b1"]).reshape(128, 1)
    p["rn2"] = A(rn["W2"])
    p["rnb2"] = A(rn["b2"]).reshape(5, 1)
    p["rnb2r"] = A(rn["b2"]).reshape(1, 5)

    # static one-hot broadcast/tile pattern
    bc = np.zeros((64, EG), dtype=np.float32)
    for a in range(PAD):
        bc[a, a * PAD : (a + 1) * PAD] = 1.0
    for b in range(PAD):
        bc[32 + b, b::PAD] = 1.0
    p["BC"] = bc

    blocks = []
    for lp in params["layers"]:
        eW1 = A(lp["e_W1"])
        nW1 = A(lp["n_W1"])
        for arr in (
            eW1[0:256],
            eW1[256:512],
            A(lp["e_W2"]),
            A(lp["c_W1"]),
            nW1[0:256] + nW1[512:768],
            nW1[256:512],
            A(lp["n_W2"]),
        ):
            blocks.append(arr[0:128, :])
            blocks.append(arr[128:256, :])
    blocks.append(A(rn["W1"])[0:128, :])
    blocks.append(A(rn["W1"])[128:256, :])
    for lp in params["layers"]:
        cw2 = A(lp["c_W2"]).reshape(HID, 1)
        blocks.append(cw2[0:128, :])
        blocks.append(cw2[128:256, :])
    for lp in params["layers"]:
        for b in (lp["e_b1"], lp["e_b2"], lp["c_b1"], lp["n_b1"], lp["n_b2"]):
            blocks.append(col2(b))
    p["big0"] = np.ascontiguousarray(np.concatenate(blocks, axis=1))
    assert p["big0"].shape == (128, 7448), p["big0"].shape

    for l, lp in enumerate(params["layers"]):
        eW1 = A(lp["e_W1"])  # [518, 256]
        p[f"ehr{l}"] = np.ascontiguousarray(eW1[0:256])
        p[f"ehc{l}"] = np.ascontiguousarray(eW1[256:512])
        wz = np.zeros((10, HID), dtype=np.float32)
        wz[4] = eW1[512]
        wz[5:10] = eW1[513:518]
        p[f"wz{l}"] = wz
        p[f"eb1_{l}"] = col2(lp["e_b1"])
        p[f"ew2_{l}"] = A(lp["e_W2"])
        p[f"eb2_{l}"] = col2(lp["e_b2"])
        p[f"cw1_{l}"] = A(lp["c_W1"])
        p[f"cb1_{l}"] = col2(lp["c_b1"])
        p[f"cw2_{l}"] = A(lp["c_W2"]).reshape(HID, 1)
        nW1 = A(lp["n_W1"])  # [768, 256]
        p[f"nw1h_{l}"] = np.ascontiguousarray(nW1[0:256] + nW1[512:768])
        p[f"nw1a_{l}"] = np.ascontiguousarray(nW1[256:512])
        p[f"nb1_{l}"] = col2(lp["n_b1"])
        p[f"nw2_{l}"] = A(lp["n_W2"])
        p[f"nb2_{l}"] = col2(lp["n_b2"])
    return p


def _expected_edges():
    i = np.arange(PAD)
    row = np.repeat(i, PAD)
    col = np.tile(i, PAD)
    offs = (np.arange(B) * PAD)[:, None]
    return np.stack(
        [
            (row[None, :] + offs).reshape(-1),
            (col[None, :] + offs).reshape(-1),
        ]
    ).astype(np.int64)


def _get_nc():
    if "nc" not in _CACHE:
        _CACHE["nc"] = _build_program()
    return _CACHE["nc"]


def _install_ntff_hook():
    """antenv.axon_hooks is absent in this image; shim it and register the
    ctypes NTFF profiling hook from trn_agent_boot so trace=True works."""
    import types
    import importlib

    try:
        importlib.import_module("antenv.axon_hooks")
        return
    except ImportError:
        pass
    try:
        from trn_agent_boot.trn_boot import _ntff_profile_via_ctypes
    except ImportError:
        return
    import antenv

    mod = types.ModuleType("antenv.axon_hooks")
    mod._hook = _ntff_profile_via_ctypes("/opt/axon/libaxon_pjrt.so")

    def set_axon_ntff_profile_hook(h):
        mod._hook = h

    def get_axon_ntff_profile_hook():
        return mod._hook

    mod.set_axon_ntff_profile_hook = set_axon_ntff_profile_hook
    mod.get_axon_ntff_profile_hook = get_axon_ntff_profile_hook
    sys.modules["antenv.axon_hooks"] = mod
    antenv.axon_hooks = mod


def kernel(z, label, edges, n_nodes, params, _trace=False):
    from concourse.bass_utils import run_bass_kernel_spmd

    z = np.asarray(z, dtype=np.float32)
    label = np.asarray(label, dtype=np.float32)
    edges_np = np.asarray(edges)
    assert int(n_nodes) == PAD, f"n_nodes must be {PAD}"
    if not np.array_equal(edges_np.astype(np.int64), _expected_edges()):
        raise ValueError("edges do not match the full per-graph grid layout")

    wmap = _prep_weights(params)
    nc = _get_nc()

    in_maps = []
    for c in range(NCORES):
        ns = slice(c * NL, (c + 1) * NL)
        m = dict(wmap)
        m["z"] = np.ascontiguousarray(z[ns])
        m["lab"] = np.ascontiguousarray(label[ns])
        in_maps.append(m)

    kw = {}
    if _trace:
        _install_ntff_hook()
        kw = dict(trace=True, trace_cores=[0])
    res = run_bass_kernel_spmd(nc, in_maps, core_ids=list(range(NCORES)), **kw)

    h_out = np.concatenate([res.results[c]["h_out"] for c in range(NCORES)], 0)
    coord = np.concatenate(
        [res.results[c]["coord_out"] for c in range(NCORES)], 0
    )
    ea = np.concatenate([res.results[c]["ea_out"] for c in range(NCORES)], 0)
    an = np.concatenate([res.results[c]["an_out"] for c in range(NCORES)], 0)
    if _trace:
        return (h_out, coord, ea, an), res
    return h_out, coord, ea, an


# revision 70
# speedup vs baseline: 1.0083x; 1.0045x over previous
"""Trainium2 Bass kernel for nn_Decoder_78176994721983 (EGNN-style decoder).

Data-parallel over graphs: 128 graphs x 30 padded nodes, sharded as 16
graphs per NeuronCore across 8 cores. All segment ops are device-local
(edges form a full 30x30 grid inside each graph, so gathers/scatters
become structured broadcast matmuls and grouped free-dim reductions).

Key algorithmic restructuring vs the reference:
  - e_in = [h[row], h[col], radial, edge_attr] @ e_W1 is decomposed into
    node-level pre-products P = h @ e_W1[:256], Q = h @ e_W1[256:512]
    plus an edge-level K=70 matmul whose stationary operand stacks
    [P_g (30 rows), 0, Q_g (30 rows at partition 32), 0, W1_radial,
    W1_edge_attr] and whose moving operand is a static one-hot
    broadcast/tile pattern with radial and edge_attr rows appended.
  - segment_sum over rows = grouped reduction over the inner 30 (col)
    axis of [*, 30a, 30b] views; cnt == 30 exactly (full grid).
  - coord update folds 1/30 into a scalar_tensor_tensor.
"""

import os
import sys

if "/opt/trn_rl_repo" not in sys.path:
    sys.path.insert(0, "/opt/trn_rl_repo")

import numpy as np

B, PAD, LAT, HID = 128, 30, 64, 256
ZIN = LAT + 7  # 71
NCORES = 8
G = B // NCORES            # graphs per core: 16
NL = G * PAD               # nodes per core: 480
EL = G * PAD * PAD         # edges per core: 14400
EG = PAD * PAD             # edges per graph: 900
N_LAYERS = 2

# matmul dtype policy: relaxed fp32 (single-pass PE mode, 4x faster at
# free-dim >= 256) for the big edge matmuls; plain fp32 otherwise.
RELAXED = os.environ.get("KERNEL_F32R", "1") != "0"

_CACHE = {}


# --------------------------------------------------------------------------
# walrus in this container accepts at most ONE sync-wait per instruction;
# Tile emits several. Split extras onto same-engine no-ops just before the
# instruction (same sequencer stream => identical blocking semantics).
def _legalize_waits(nc, mybir):
    n_split = 0
    for f in nc.m.functions:
        for blk in f.blocks:
            insts = list(blk.instructions)
            out = []
            changed = False
            for inst in insts:
                si = inst.sync_info
                if si is not None:
                    waits = list(si.on_wait)
                    if len(waits) > 1:
                        changed = True
                        n_split += 1
                        for j, w in enumerate(waits[:-1]):
                            out.append(
                                mybir.InstNoOp(
                                    name=f"{inst.name}-w{j}",
                                    sync_info=mybir.SyncInfo(
                                        on_wait=[w], on_update=[]
                                    ),
                                    bass_nofuse=True,
                                    engine=inst.engine,
                                )
                            )
                        si.on_wait = waits[-1:]
                out.append(inst)
            if changed:
                blk.instructions = out
    return n_split


def _build_program(legalize=True):
    import concourse.bass as bass
    import concourse.mybir as mybir
    import concourse.tile as tile

    f32 = mybir.dt.float32
    f32r = mybir.dt.float32r
    AF = mybir.ActivationFunctionType
    ALU = mybir.AluOpType
    AX = mybir.AxisListType

    def rx(ap):
        return ap.bitcast(f32r) if RELAXED else ap

    # The BIR verifier requires every producer of an fp32r-matmul operand
    # to emit fp32r-rounded output: write those tensors through fp32r-
    # bitcast views (engines round on the output cast). Non-matmul readers
    # keep plain f32 views of the same bits.
    rw = rx

    nc = bass.Bass()

    def din(name, shape):
        return nc.dram_tensor(name, list(shape), f32, kind="ExternalInput")

    def dout(name, shape):
        return nc.dram_tensor(name, list(shape), f32, kind="ExternalOutput")

    z_d = din("z", [NL, LAT])
    lab_d = din("lab", [NL, 7])
    bc_d = din("BC", [64, EG])
    embw_d = din("embw", [ZIN, HID])
    embb_d = din("embb", [128, 2])
    rp1_d = din("rp1", [ZIN, 15])
    rpb1_d = din("rpb1", [15, 1])
    rp2_d = din("rp2", [15, 3])
    rpb2_d = din("rpb2", [3, 1])
    re1_d = din("re1", [ZIN, 30 * 60])
    reb1_d = din("reb1", [60, 1])
    re2b_d = din("re2b", [61, 4500])
    re2c_d = din("re2c", [61, 4500])
    ones16_d = din("ones16", [1, 128])
    rnb2r_d = din("rnb2r", [1, 5])
    an1_d = din("an1", [ZIN, 30 * 60])
    anb1_d = din("anb1", [60, 1])
    an2_d = din("an2", [60, PAD])
    anb2_d = din("anb2", [PAD, 1])
    idm_d = din("idm", [128, 128])
    rn1_d = din("rn1", [HID, 128])
    big0_d = din("big0", [128, 7448])
    rnb1_d = din("rnb1", [128, 1])
    rn2_d = din("rn2", [128, 5])
    rnb2_d = din("rnb2", [5, 1])

    L = []
    for l in range(N_LAYERS):
        L.append(
            {
                "ehr": din(f"ehr{l}", [HID, HID]),
                "ehc": din(f"ehc{l}", [HID, HID]),
                "wz": din(f"wz{l}", [10, HID]),
                "eb1": din(f"eb1_{l}", [128, 2]),
                "ew2": din(f"ew2_{l}", [HID, HID]),
                "eb2": din(f"eb2_{l}", [128, 2]),
                "cw1": din(f"cw1_{l}", [HID, HID]),
                "cb1": din(f"cb1_{l}", [128, 2]),
                "cw2": din(f"cw2_{l}", [HID, 1]),
                "nw1h": din(f"nw1h_{l}", [HID, HID]),
                "nw1a": din(f"nw1a_{l}", [HID, HID]),
                "nb1": din(f"nb1_{l}", [128, 2]),
                "nw2": din(f"nw2_{l}", [HID, HID]),
                "nb2": din(f"nb2_{l}", [128, 2]),
            }
        )

    hout_d = dout("h_out", [NL, 5])
    coord_d = dout("coord_out", [NL, 3])
    ea_d = dout("ea_out", [EL, 5])
    an_d = dout("an_out", [G, PAD])

    with tile.TileContext(nc) as tc:
        with (
            tc.tile_pool(name="wb", bufs=1) as W,
            tc.tile_pool(name="sb", bufs=2) as SB,
            tc.tile_pool(name="ep", bufs=2, space="PSUM") as EP,
            tc.tile_pool(name="ep2", bufs=5, space="PSUM") as EP2,
            tc.tile_pool(name="wp", bufs=1, space="PSUM") as WP,
        ):
            # ---------------- static loads ----------------
            # Order matters: DMA queues are FIFO, so the tiny inputs that
            # gate the whole dependency tree (z/lab -> z_t transposes ->
            # heads/h/coord) go first, then the weights by first use,
            # with the layer-1 block last.
            idm_t = W.tile([128, 128], f32, tag="idm")
            nc.sync.dma_start(idm_t[:], idm_d.ap())
            z_t = W.tile([ZIN, NL], f32, tag="z_t")
            zls = []
            for c in range(4):
                nsl = slice(c * 120, (c + 1) * 120)
                zl = SB.tile([120, ZIN], f32, tag="zl", name=f"zl{c}")
                nc.sync.dma_start(zl[:, 0:LAT], z_d.ap()[nsl, :])
                nc.scalar.dma_start(zl[:, LAT:ZIN], lab_d.ap()[nsl, :])
                zls.append(zl)
            for c in range(4):
                nsl = slice(c * 120, (c + 1) * 120)
                zps = EP.tile([ZIN, 120], f32, tag="ep", name=f"zps{c}")
                nc.tensor.transpose(zps[:], zls[c][:], idm_t[0:120, 0:120])
                nc.vector.tensor_copy(z_t[:, nsl], zps[:])
            # re2b first: it gates y2_rm -> re6 edge_attr rows -> the very
            # first pre matmul of layer 0 (longest startup pole).
            re2b_t = W.tile([61, 4500], f32, tag="re2b")
            for ci, e in zip(range(3), (nc.scalar, nc.sync, nc.scalar)):
                csl = slice(ci * 1500, (ci + 1) * 1500)
                e.dma_start(rw(re2b_t[:, csl]), rw(re2b_d.ap()[:, csl]))
            big0_t = W.tile([128, 7448], f32, tag="big0")
            nc.sync.dma_start(rw(big0_t[:, 0:1024]), rw(big0_d.ap()[:, 0:1024]))
            nc.scalar.dma_start(
                rw(big0_t[:, 7168:7448]), rw(big0_d.ap()[:, 7168:7448])
            )
            nc.sync.dma_start(
                rw(big0_t[:, 1024:1536]), rw(big0_d.ap()[:, 1024:1536])
            )
            nc.scalar.dma_start(
                rw(big0_t[:, 1536:2048]), rw(big0_d.ap()[:, 1536:2048])
            )
            nc.sync.dma_start(
                rw(big0_t[:, 2048:3584]), rw(big0_d.ap()[:, 2048:3584])
            )
            # zg chunks: zg^T rows [a*71 .. a*71+71) for graph g are just
            # z_t columns g*30+a -> strided slices of z_t, no extra tile.
            zg3 = z_t[:].rearrange("j (g a) -> j a g", a=PAD)
            # Static one-hot broadcast/tile pattern, shared by all graphs.
            # The per-graph radial/edge_attr rows live in a separate
            # [70, 900] tile (rows 64..69) so the pre matmul is
            # K=64 (static) + K=6 (per graph) accumulating in PSUM.
            re6s = []
            for i in range(2):
                r = W.tile([70, EG], f32, tag=f"re6_{i}", name=f"re6s{i}")
                if i == 0:
                    # graph 0's tile gates the first pre matmul: fast
                    # split load on both HWDGE queues
                    nc.sync.dma_start(rw(r[0:32, :]), rw(bc_d.ap()[0:32, :]))
                    nc.scalar.dma_start(rw(r[32:64, :]), rw(bc_d.ap()[32:64, :]))
                else:
                    # off the critical path (first used at graph 1)
                    nc.gpsimd.dma_start(rw(r[0:64, :]), rw(bc_d.ap()))
                re6s.append(r)

            _eng_rot = [nc.sync, nc.scalar, nc.gpsimd]
            _eng_i = [0]

            def _eng():
                e = _eng_rot[_eng_i[0] % len(_eng_rot)]
                _eng_i[0] += 1
                return e

            def wload(d, shape, tag):
                t = W.tile(list(shape), f32, tag=tag)
                _eng().dma_start(t[:], d.ap())
                return t

            embw_t = wload(embw_d, [ZIN, HID], "embw")
            embb_t = wload(embb_d, [128, 2], "embb")
            rp1_t = wload(rp1_d, [ZIN, 15], "rp1")
            rpb1_t = wload(rpb1_d, [15, 1], "rpb1")
            rp2_t = wload(rp2_d, [15, 3], "rp2")
            rpb2_t = wload(rpb2_d, [3, 1], "rpb2")
            re1_t = wload(re1_d, [ZIN, 1800], "re1")
            reb1_t = wload(reb1_d, [60, 1], "reb1")
            re2c_t = W.tile([61, 4500], f32, tag="re2c")
            nc.gpsimd.dma_start(rw(re2c_t[:]), rw(re2c_d.ap()))
            an1_t = wload(an1_d, [ZIN, 1800], "an1")
            anb1_t = wload(anb1_d, [60, 1], "anb1")
            an2_t = wload(an2_d, [60, PAD], "an2")
            anb2_t = wload(anb2_d, [PAD, 1], "anb2")
            rnb1_t = wload(rnb1_d, [128, 1], "rnb1")
            rn2_t = W.tile([128, 5], f32, tag="rn2")
            nc.sync.dma_start(rn2_t[:], rn2_d.ap())
            ones128_t = W.tile([1, 128], f32, tag="ones128")
            nc.scalar.dma_start(ones128_t[:], ones16_d.ap())
            rnb2r_t = W.tile([1, 5], f32, tag="rnb2r")
            nc.scalar.dma_start(rnb2r_t[:], rnb2r_d.ap())
            rn1_t = None  # assigned after the packed load below

            # all [128, x] layer weights packed host-side into one tensor
            # -> a single large contiguous DMA instead of ~32 small ones
            # deferred layer-1 weights (consumed ~halfway through)
            nc.sync.dma_start(
                rw(big0_t[:, 3584:7168]), rw(big0_d.ap()[:, 3584:7168])
            )
            _off = [0]

            def _blk(cols):
                o = _off[0]
                _off[0] += cols
                return big0_t[:, o : o + cols]

            LT = []
            for l in range(N_LAYERS):
                d = L[l]
                t = {}
                for nm in ("ehr", "ehc", "ew2", "cw1", "nw1h", "nw1a", "nw2"):
                    t[nm] = [_blk(HID), _blk(HID)]
                t["wz"] = d["wz"]  # stays in DRAM; DMA'd per graph
                LT.append(t)
            rn1_p = [_blk(128), _blk(128)]
            for l in range(N_LAYERS):
                LT[l]["cw2"] = [_blk(1), _blk(1)]
            for l in range(N_LAYERS):
                for nm in ("eb1", "eb2", "cb1", "nb1", "nb2"):
                    LT[l][nm] = _blk(2)
            rn1_t = rn1_p

            # ---------------- graph-level heads ----------------
            # re_edge hidden: y1 = silu(zg @ W1 + b1), K accumulated in
            # 30 chunks of 71 (one per node slot a).
            y1p = EP.tile([60, G], f32, tag="ep")
            for a in range(PAD):
                nc.tensor.matmul(
                    y1p[:],
                    re1_t[:, a * 60 : (a + 1) * 60],
                    zg3[:, a, :],
                    start=(a == 0),
                    stop=(a == PAD - 1),
                )
            # [y1; ones] so the next matmul folds the bias in
            y1s = SB.tile([61, G], f32, tag="y1s")
            nc.scalar.activation(
                rw(y1s[0:60, :]), y1p[:], AF.Silu, bias=reb1_t[:, 0:1]
            )
            nc.sync.dma_start(rw(y1s[60:61, :]), rw(ones16_d.ap()[:, 0:G]))
            # atom_num hidden
            ay1p = EP.tile([60, G], f32, tag="ep")
            for a in range(PAD):
                nc.tensor.matmul(
                    ay1p[:],
                    an1_t[:, a * 60 : (a + 1) * 60],
                    zg3[:, a, :],
                    start=(a == 0),
                    stop=(a == PAD - 1),
                )
            ay1s = SB.tile([60, G], f32, tag="ay1s")
            nc.scalar.activation(ay1s[:], ay1p[:], AF.Silu, bias=anb1_t[:, 0:1])
            # re_edge out, graph-major with host-permuted columns:
            # y2_rm[g, j*900+k] = edge_attr[g*900+k, j]; bias folded via
            # the ones row of y1s.
            y2_rm = SB.tile([G, 4500], f32, tag="y2_rm", bufs=1)
            for fb in range(9):
                fsl = slice(fb * 500, (fb + 1) * 500)
                y2p = EP.tile([G, 500], f32, tag="ep", name=f"y2p{fb}")
                nc.tensor.matmul(
                    y2p[:], rx(y1s[:]), rx(re2b_t[:, fsl]),
                    start=True, stop=True,
                )
                nc.vector.tensor_copy(y2_rm[:, fsl], y2p[:])
            # edge_attr output: second head pass with unpermuted columns
            # so the DRAM write is fully contiguous (16 descriptors/block)
            ea2d = ea_d.ap().rearrange("e j -> (e j)").rearrange(
                "(g f) -> g f", g=G
            )
            for fb in range(9):
                fsl = slice(fb * 500, (fb + 1) * 500)
                eap = EP.tile([G, 500], f32, tag="ep", name=f"eap{fb}")
                nc.tensor.matmul(
                    eap[:], rx(y1s[:]), rx(re2c_t[:, fsl]),
                    start=True, stop=True,
                )
                eas = SB.tile([G, 500], f32, tag="eas", bufs=2, name=f"eas{fb}")
                nc.vector.tensor_copy(eas[:], eap[:])
                nc.sync.dma_start(ea2d[:, fsl], eas[:])
            # atom_num out
            ayp = EP.tile([PAD, G], f32, tag="ep")
            nc.tensor.matmul(ayp[:], an2_t[:], ay1s[:], start=True, stop=True)
            ay_sb = SB.tile([PAD, G], f32, tag="ay_sb")
            nc.scalar.activation(
                ay_sb[:], ayp[:], AF.Identity, bias=anb2_t[:, 0:1]
            )
            nc.sync.dma_start(
                an_d.ap().rearrange("g k -> k g"), ay_sb[:]
            )


            # ---------------- h / coord init ----------------
            h_cur = []
            for fb in range(2):
                hp = EP.tile([128, NL], f32, tag="ep")
                nc.tensor.matmul(
                    hp[:], embw_t[:, fb * 128 : (fb + 1) * 128], z_t[:],
                    start=True, stop=True,
                )
                ht = SB.tile([128, NL], f32, tag=f"h{fb}")
                nc.scalar.activation(
                    rw(ht[:]), hp[:], AF.Identity, bias=embb_t[:, fb : fb + 1]
                )
                h_cur.append(ht)
            cp1 = EP.tile([15, NL], f32, tag="ep")
            nc.tensor.matmul(cp1[:], rp1_t[:], z_t[:], start=True, stop=True)
            c1s = SB.tile([15, NL], f32, tag="c1s")
            nc.scalar.activation(c1s[:], cp1[:], AF.Silu, bias=rpb1_t[:, 0:1])
            cp2 = EP.tile([3, NL], f32, tag="ep")
            nc.tensor.matmul(cp2[:], rp2_t[:], c1s[:], start=True, stop=True)
            coordT = SB.tile([3, NL], f32, tag="coordT")
            nc.scalar.activation(
                coordT[:], cp2[:], AF.Identity, bias=rpb2_t[:, 0:1]
            )
            # coordS[g, d*30+a] = coord[g*30+a, d]
            coordS = SB.tile([G, 3 * PAD], f32, tag="coordS", bufs=1)
            for d in range(3):
                # shapes differ but iteration orders match: (g, a) vs g*30+a
                nc.sync.dma_start(
                    coordS[:, d * PAD : (d + 1) * PAD],
                    coordT[d : d + 1, :],
                )

            # ---------------- message-passing layers ----------------
            for l in range(N_LAYERS):
                t = LT[l]
                # coordinate differences, stacked graphs on partitions
                cd = []
                for d in range(3):
                    cdt = SB.tile([G, EG], f32, tag=f"cd{d}", bufs=1)
                    ca = coordS[:, d * PAD : (d + 1) * PAD]
                    nc.vector.tensor_sub(
                        cdt[:].rearrange("g (a b) -> g a b", b=PAD),
                        ca.broadcast_to([G, PAD, PAD]),
                        ca[:, None, :].broadcast_to([G, PAD, PAD]),
                    )
                    cd.append(cdt)
                radial = SB.tile([G, EG], f32, tag="radial", bufs=1)
                sqt = SB.tile([G, EG], f32, tag="trans", bufs=1)
                nc.vector.tensor_mul(radial[:], cd[0][:], cd[0][:])
                nc.vector.tensor_mul(sqt[:], cd[1][:], cd[1][:])
                nc.vector.tensor_add(radial[:], radial[:], sqt[:])
                nc.vector.tensor_mul(sqt[:], cd[2][:], cd[2][:])
                nc.vector.tensor_add(radial[:], radial[:], sqt[:])
                agg = [
                    SB.tile([128, NL], f32, tag=f"agg{fb}", bufs=1, name=f"agg{l}_{fb}")
                    for fb in range(2)
                ]
                w_all = SB.tile([G, EG], f32, tag="w_all", bufs=1)

                for g in range(G):
                    ns = slice(g * PAD, (g + 1) * PAD)
                    # P and Q node-level pre-products, both M=30 at (0,0)
                    # (the fp32r ISA check rejects col-offset tile_position,
                    # so Q reaches s70 rows 32..61 via a staging DMA).
                    psP = EP.tile([30, HID], f32, tag="ep", name=f"psP{l}_{g}")
                    for kc in range(2):
                        nc.tensor.matmul(
                            psP[:],
                            rx(h_cur[kc][:, ns]),
                            rx(t["ehr"][kc][:]),
                            start=(kc == 0),
                            stop=(kc == 1),
                        )
                    psQ = EP.tile([30, HID], f32, tag="ep", name=f"psQ{l}_{g}")
                    for kc in range(2):
                        nc.tensor.matmul(
                            psQ[:],
                            rx(h_cur[kc][:, ns]),
                            rx(t["ehc"][kc][:]),
                            start=(kc == 0),
                            stop=(kc == 1),
                        )
                    s70 = SB.tile([70, HID], f32, tag="s70", bufs=4)
                    nc.sync.dma_start(rw(s70[30:32, :]), rw(t["wz"].ap()[0:2, :]))
                    nc.sync.dma_start(rw(s70[62:70, :]), rw(t["wz"].ap()[2:10, :]))
                    nc.vector.tensor_copy(rw(s70[0:30, :]), psP[:])
                    qst = SB.tile([30, HID], f32, tag="qst", bufs=3, name=f"qst{l}_{g}")
                    nc.vector.tensor_copy(rw(qst[:]), psQ[:])
                    nc.sync.dma_start(rw(s70[32:62, :]), rw(qst[:]))
                    # per-graph radial + edge_attr rows at partitions
                    # 64..69 of the static ping-pong tile (BC rows 0..63
                    # were filled once at setup -> single K=70 pre matmul)
                    re6 = re6s[g % 2]
                    nc.sync.dma_start(rw(re6[64:65, :]), rw(radial[g : g + 1, :]))
                    for j in range(5):
                        nc.sync.dma_start(
                            rw(re6[65 + j : 66 + j, :]),
                            rw(y2_rm[g : g + 1, j * 900 : (j + 1) * 900]),
                        )

                    wv_sb = SB.tile([1, EG], f32, tag="wv_sb", bufs=1, name=f"wv_sb{l}_{g}")
                    for hh in range(2):
                        cs = slice(hh * 450, (hh + 1) * 450)
                        # edge MLP stage 1 (pre = e_in @ e_W1)
                        m1 = []
                        for fb in range(2):
                            fs = slice(fb * 128, (fb + 1) * 128)
                            pre = EP2.tile([128, 450], f32, tag="ep2")
                            nc.tensor.matmul(
                                pre[:], rx(s70[:, fs]), rx(re6[:, cs]),
                                start=True, stop=True,
                            )
                            m1t = SB.tile([128, 450], f32, tag=f"m1_{fb}", bufs=4)
                            nc.scalar.activation(
                                rw(m1t[:]), pre[:], AF.Silu,
                                bias=t["eb1"][:, fb : fb + 1],
                            )
                            m1.append(m1t)
                        # edge MLP stage 2 (m = silu(m1 @ e_W2 + b2))
                        mg = []
                        for fb in range(2):
                            fs = slice(fb * 128, (fb + 1) * 128)
                            pre2 = EP2.tile([128, 450], f32, tag="ep2")
                            for kc in range(2):
                                nc.tensor.matmul(
                                    pre2[:], rx(t["ew2"][kc][:, fs]),
                                    rx(m1[kc][:]),
                                    start=(kc == 0), stop=(kc == 1),
                                )
                            mt = SB.tile([128, 450], f32, tag=f"m_{fb}", bufs=4)
                            nc.scalar.activation(
                                rw(mt[:]), pre2[:], AF.Silu,
                                bias=t["eb2"][:, fb : fb + 1],
                            )
                            mg.append(mt)
                        # coord gate hidden
                        mw = []
                        for fb in range(2):
                            fs = slice(fb * 128, (fb + 1) * 128)
                            prew = EP2.tile([128, 450], f32, tag="ep2")
                            for kc in range(2):
                                nc.tensor.matmul(
                                    prew[:], rx(t["cw1"][kc][:, fs]),
                                    rx(mg[kc][:]),
                                    start=(kc == 0), stop=(kc == 1),
                                )
                            mwt = SB.tile([128, 450], f32, tag=f"mw_{fb}")
                            nc.scalar.activation(
                                rw(mwt[:]), prew[:], AF.Silu,
                                bias=t["cb1"][:, fb : fb + 1],
                            )
                            mw.append(mwt)
                        # w = mw @ c_W2 -> [1, 450] (one psum bank per half)
                        wv = WP.tile([1, 450], f32, tag="wp", name=f"wv{l}_{g}_{hh}")
                        for kc in range(2):
                            nc.tensor.matmul(
                                wv[:], rx(t["cw2"][kc][:]), rx(mw[kc][:]),
                                start=(kc == 0), stop=(kc == 1),
                            )
                        nc.vector.tensor_copy(wv_sb[:, cs], wv[:])
                        # agg[f, a] += over this half's 15 a-rows
                        for fb in range(2):
                            with nc.allow_low_precision("f32r agg feeds f32r matmul"):
                                nc.vector.tensor_reduce(
                                    rw(agg[fb][:, g * PAD + hh * 15 : g * PAD + (hh + 1) * 15]),
                                    mg[fb][:].rearrange("p (a b) -> p a b", b=PAD),
                                    axis=AX.X,
                                    op=ALU.add,
                                )
                    # DVE cannot shift partitions: wv_sb staged on partition
                    # 0, DMA moves it to w_all row g.
                    nc.sync.dma_start(w_all[g : g + 1, :], wv_sb[:])

                # coord update: coordS += (sum_b cd*w) / 30
                for d in range(3):
                    trans = SB.tile([G, EG], f32, tag="trans", bufs=1, name=f"trans{l}_{d}")
                    nc.vector.tensor_mul(trans[:], cd[d][:], w_all[:])
                    ssum = SB.tile([G, PAD], f32, tag="ssum", bufs=1, name=f"ssum{l}_{d}")
                    nc.vector.tensor_reduce(
                        ssum[:],
                        trans[:].rearrange("g (a b) -> g a b", b=PAD),
                        axis=AX.X,
                        op=ALU.add,
                    )
                    nc.vector.scalar_tensor_tensor(
                        coordS[:, d * PAD : (d + 1) * PAD],
                        ssum[:],
                        1.0 / PAD,
                        coordS[:, d * PAD : (d + 1) * PAD],
                        op0=ALU.mult,
                        op1=ALU.add,
                    )

                # node update
                s1 = []
                for fb in range(2):
                    hn = EP.tile([128, NL], f32, tag="ep")
                    for kc in range(2):
                        nc.tensor.matmul(
                            hn[:],
                            rx(t["nw1h"][kc][:, fb * 128 : (fb + 1) * 128]),
                            rx(h_cur[kc][:]),
                            start=(kc == 0),
                            stop=False,
                        )
                    for kc in range(2):
                        nc.tensor.matmul(
                            hn[:],
                            rx(t["nw1a"][kc][:, fb * 128 : (fb + 1) * 128]),
                            rx(agg[kc][:]),
                            start=False,
                            stop=(kc == 1),
                        )
                    s1t = SB.tile([128, NL], f32, tag=f"s1_{fb}", bufs=1)
                    nc.scalar.activation(
                        rw(s1t[:]), hn[:], AF.Silu, bias=t["nb1"][:, fb : fb + 1]
                    )
                    s1.append(s1t)
                if l < N_LAYERS - 1:
                    h_new = []
                    for fb in range(2):
                        hp2 = EP.tile([128, NL], f32, tag="ep")
                        for kc in range(2):
                            nc.tensor.matmul(
                                hp2[:],
                                rx(t["nw2"][kc][:, fb * 128 : (fb + 1) * 128]),
                                rx(s1[kc][:]),
                                start=(kc == 0),
                                stop=(kc == 1),
                            )
                        ht = SB.tile([128, NL], f32, tag=f"h{fb}")
                        nc.scalar.activation(
                            rw(ht[:]), hp2[:], AF.Identity,
                            bias=t["nb2"][:, fb : fb + 1],
                        )
                        h_new.append(ht)
                    h_cur = h_new
                else:
                    # last layer: h is only consumed by the h_out head, so
                    # n_W2/n_b2 are composed into rn1/rnb1 host-side and the
                    # head reads s1 directly.
                    s1_last = s1

            # ---------------- output heads ----------------
            # coord first so its (scattered) store overlaps the h_out head
            coord3 = coord_d.ap().rearrange("(g a) d -> g d a", a=PAD)
            for d, e in zip(range(3), (nc.sync, nc.scalar, nc.gpsimd)):
                e.dma_start(
                    coord3[:, d, :], coordS[:, d * PAD : (d + 1) * PAD]
                )
            r1p = EP.tile([128, NL], f32, tag="ep")
            for kc in range(2):
                nc.tensor.matmul(
                    r1p[:], rx(rn1_t[kc][:]), rx(s1_last[kc][:]),
                    start=(kc == 0), stop=(kc == 1),
                )
            r1s = SB.tile([128, NL], f32, tag="r1s")
            nc.scalar.activation(r1s[:], r1p[:], AF.Silu, bias=rnb1_t[:, 0:1])
            # node-major blocks: contiguous h_out rows, bias via ones row
            for c in range(4):
                nbl = slice(c * 120, (c + 1) * 120)
                hop = EP.tile([120, 5], f32, tag="ep", name=f"hop{c}")
                nc.tensor.matmul(
                    hop[:], r1s[:, nbl], rn2_t[:],
                    start=True, stop=False,
                )
                nc.tensor.matmul(
                    hop[:], ones128_t[:, 0:120], rnb2r_t[:],
                    start=False, stop=True,
                )
                hos = SB.tile([120, 5], f32, tag="hos", bufs=2, name=f"hos{c}")
                nc.vector.tensor_copy(hos[:], hop[:])
                nc.sync.dma_start(hout_d.ap()[nbl, :], hos[:])

    import concourse.mybir as mybir  # noqa: F811

    if legalize:
        _legalize_waits(nc, mybir)
    return nc


def _prep_weights(params):
    """Host-side weight massaging -> dict of named f32 arrays."""
    def A(x):
        return np.ascontiguousarray(np.asarray(x), dtype=np.float32)

    def col2(b):  # [256] -> [128, 2], column fb = features fb*128..
        return np.ascontiguousarray(A(b).reshape(2, 128).T)

    p = {}
    p["embw"] = A(params["emb_W"])
    p["embb"] = col2(params["emb_b"])
    rp = params["re_pos"]
    p["rp1"] = A(rp["W1"])
    p["rpb1"] = A(rp["b1"]).reshape(15, 1)
    p["rp2"] = A(rp["W2"])
    p["rpb2"] = A(rp["b2"]).reshape(3, 1)
    re = params["re_edge"]
    p["re1"] = np.ascontiguousarray(
        A(re["W1"]).reshape(PAD, ZIN, 60).transpose(1, 0, 2).reshape(ZIN, PAD * 60)
    )
    p["reb1"] = A(re["b1"]).reshape(60, 1)
    W2p = A(re["W2"]).reshape(60, 900, 5).transpose(0, 2, 1).reshape(60, 4500)
    b2p = A(re["b2"]).reshape(900, 5).T.reshape(1, 4500)
    p["re2b"] = np.ascontiguousarray(np.concatenate([W2p, b2p], axis=0))
    p["re2c"] = np.ascontiguousarray(
        np.concatenate([A(re["W2"]), A(re["b2"]).reshape(1, 4500)], axis=0)
    )
    p["ones16"] = np.ones((1, 128), dtype=np.float32)
    an = params["atom_num"]
    p["an1"] = np.ascontiguousarray(
        A(an["W1"]).reshape(PAD, ZIN, 60).transpose(1, 0, 2).reshape(ZIN, PAD * 60)
    )
    p["anb1"] = A(an["b1"]).reshape(60, 1)
    p["an2"] = A(an["W2"])
    p["anb2"] = A(an["b2"]).reshape(PAD, 1)
    rn = params["re_nodes"]
    p["idm"] = np.eye(128, dtype=np.float32)
    p["rn1"] = A(rn["W1"])
    lpL0 = params["layers"][-1]
    rnb1c = A(rn["b1"]).astype(np.float64) + A(lpL0["n_b2"]).astype(
        np.float64
    ) @ A(rn["W1"]).astype(np.float64)
    p["rnb1"] = rnb1c.astype(np.float32).reshape(128, 1)
    p["rn2"] = A(rn["W2"])
    p["rnb2"] = A(rn["b2"]).reshape(5, 1)
    p["rnb2r"] = A(rn["b2"]).reshape(1, 5)

    # static one-hot broadcast/tile pattern
    bc = np.zeros((64, EG), dtype=np.float32)
    for a in range(PAD):
        bc[a, a * PAD : (a + 1) * PAD] = 1.0
    for b in range(PAD):
        bc[32 + b, b::PAD] = 1.0
    p["BC"] = bc

    blocks = []
    for lp in params["layers"]:
        eW1 = A(lp["e_W1"])
        nW1 = A(lp["n_W1"])
        for arr in (
            eW1[0:256],
            eW1[256:512],
            A(lp["e_W2"]),
            A(lp["c_W1"]),
            nW1[0:256] + nW1[512:768],
            nW1[256:512],
            A(lp["n_W2"]),
        ):
            blocks.append(arr[0:128, :])
            blocks.append(arr[128:256, :])
    lpL = params["layers"][-1]
    rn_w1 = A(rn["W1"])
    rnc = (A(lpL["n_W2"]).astype(np.float64) @ rn_w1.astype(np.float64)).astype(
        np.float32
    )
    blocks.append(rnc[0:128, :])
    blocks.append(rnc[128:256, :])
    for lp in params["layers"]:
        cw2 = A(lp["c_W2"]).reshape(HID, 1)
        blocks.append(cw2[0:128, :])
        blocks.append(cw2[128:256, :])
    for lp in params["layers"]:
        for b in (lp["e_b1"], lp["e_b2"], lp["c_b1"], lp["n_b1"], lp["n_b2"]):
            blocks.append(col2(b))
    p["big0"] = np.ascontiguousarray(np.concatenate(blocks, axis=1))
    assert p["big0"].shape == (128, 7448), p["big0"].shape

    for l, lp in enumerate(params["layers"]):
        eW1 = A(lp["e_W1"])  # [518, 256]
        p[f"ehr{l}"] = np.ascontiguousarray(eW1[0:256])
        p[f"ehc{l}"] = np.ascontiguousarray(eW1[256:512])
        wz = np.zeros((10, HID), dtype=np.float32)
        wz[4] = eW1[512]
        wz[5:10] = eW1[513:518]
        p[f"wz{l}"] = wz
        p[f"eb1_{l}"] = col2(lp["e_b1"])
        p[f"ew2_{l}"] = A(lp["e_W2"])
        p[f"eb2_{l}"] = col2(lp["e_b2"])
        p[f"cw1_{l}"] = A(lp["c_W1"])
        p[f"cb1_{l}"] = col2(lp["c_b1"])
        p[f"cw2_{l}"] = A(lp["c_W2"]).reshape(HID, 1)
        nW1 = A(lp["n_W1"])  # [768, 256]
        p[f"nw1h_{l}"] = np.ascontiguousarray(nW1[0:256] + nW1[512:768])
        p[f"nw1a_{l}"] = np.ascontiguousarray(nW1[256:512])
        p[f"nb1_{l}"] = col2(lp["n_b1"])
        p[f"nw2_{l}"] = A(lp["n_W2"])
        p[f"nb2_{l}"] = col2(lp["n_b2"])
    return p


def _expected_edges():
    i = np.arange(PAD)
    row = np.repeat(i, PAD)
    col = np.tile(i, PAD)
    offs = (np.arange(B) * PAD)[:, None]
    return np.stack(
        [
            (row[None, :] + offs).reshape(-1),
            (col[None, :] + offs).reshape(-1),
        ]
    ).astype(np.int64)


def _get_nc():
    if "nc" not in _CACHE:
        _CACHE["nc"] = _build_program()
    return _CACHE["nc"]


def _install_ntff_hook():
    """antenv.axon_hooks is absent in this image; shim it and register the
    ctypes NTFF profiling hook from trn_agent_boot so trace=True works."""
    import types
    import importlib

    try:
        importlib.import_module("antenv.axon_hooks")
        return
    except ImportError:
        pass
    try:
        from trn_agent_boot.trn_boot import _ntff_profile_via_ctypes
    except ImportError:
        return
    import antenv

    mod = types.ModuleType("antenv.axon_hooks")
    mod._hook = _ntff_profile_via_ctypes("/opt/axon/libaxon_pjrt.so")

    def set_axon_ntff_profile_hook(h):
        mod._hook = h

    def get_axon_ntff_profile_hook():
        return mod._hook

    mod.set_axon_ntff_profile_hook = set_axon_ntff_profile_hook
    mod.get_axon_ntff_profile_hook = get_axon_ntff_profile_hook
    sys.modules["antenv.axon_hooks"] = mod
    antenv.axon_hooks = mod


def kernel(z, label, edges, n_nodes, params, _trace=False):
    from concourse.bass_utils import run_bass_kernel_spmd

    z = np.asarray(z, dtype=np.float32)
    label = np.asarray(label, dtype=np.float32)
    edges_np = np.asarray(edges)
    assert int(n_nodes) == PAD, f"n_nodes must be {PAD}"
    if not np.array_equal(edges_np.astype(np.int64), _expected_edges()):
        raise ValueError("edges do not match the full per-graph grid layout")

    wmap = _prep_weights(params)
    nc = _get_nc()

    in_maps = []
    for c in range(NCORES):
        ns = slice(c * NL, (c + 1) * NL)
        m = dict(wmap)
        m["z"] = np.ascontiguousarray(z[ns])
        m["lab"] = np.ascontiguousarray(label[ns])
        in_maps.append(m)

    kw = {}
    if _trace:
        _install_ntff_hook()
        kw = dict(trace=True, trace_cores=[0])
    res = run_bass_kernel_spmd(nc, in_maps, core_ids=list(range(NCORES)), **kw)

    h_out = np.concatenate([res.results[c]["h_out"] for c in range(NCORES)], 0)
    coord = np.concatenate(
        [res.results[c]["coord_out"] for c in range(NCORES)], 0
    )
    ea = np.concatenate([res.results[c]["ea_out"] for c in range(NCORES)], 0)
    an = np.concatenate([res.results[c]["an_out"] for c in range(NCORES)], 0)
    if _trace:
        return (h_out, coord, ea, an), res
    return h_out, coord, ea, an


# revision 72
# speedup vs baseline: 1.0120x; 1.0036x over previous
"""Trainium2 Bass kernel for nn_Decoder_78176994721983 (EGNN-style decoder).

Data-parallel over graphs: 128 graphs x 30 padded nodes, sharded as 16
graphs per NeuronCore across 8 cores. All segment ops are device-local
(edges form a full 30x30 grid inside each graph, so gathers/scatters
become structured broadcast matmuls and grouped free-dim reductions).

Key algorithmic restructuring vs the reference:
  - e_in = [h[row], h[col], radial, edge_attr] @ e_W1 is decomposed into
    node-level pre-products P = h @ e_W1[:256], Q = h @ e_W1[256:512]
    plus an edge-level K=70 matmul whose stationary operand stacks
    [P_g (30 rows), 0, Q_g (30 rows at partition 32), 0, W1_radial,
    W1_edge_attr] and whose moving operand is a static one-hot
    broadcast/tile pattern with radial and edge_attr rows appended.
  - segment_sum over rows = grouped reduction over the inner 30 (col)
    axis of [*, 30a, 30b] views; cnt == 30 exactly (full grid).
  - coord update folds 1/30 into a scalar_tensor_tensor.
"""

import os
import sys

if "/opt/trn_rl_repo" not in sys.path:
    sys.path.insert(0, "/opt/trn_rl_repo")

import numpy as np

B, PAD, LAT, HID = 128, 30, 64, 256
ZIN = LAT + 7  # 71
NCORES = 8
G = B // NCORES            # graphs per core: 16
NL = G * PAD               # nodes per core: 480
EL = G * PAD * PAD         # edges per core: 14400
EG = PAD * PAD             # edges per graph: 900
N_LAYERS = 2

# matmul dtype policy: relaxed fp32 (single-pass PE mode, 4x faster at
# free-dim >= 256) for the big edge matmuls; plain fp32 otherwise.
RELAXED = os.environ.get("KERNEL_F32R", "1") != "0"

_CACHE = {}


# --------------------------------------------------------------------------
# walrus in this container accepts at most ONE sync-wait per instruction;
# Tile emits several. Split extras onto same-engine no-ops just before the
# instruction (same sequencer stream => identical blocking semantics).
def _legalize_waits(nc, mybir):
    n_split = 0
    for f in nc.m.functions:
        for blk in f.blocks:
            insts = list(blk.instructions)
            out = []
            changed = False
            for inst in insts:
                si = inst.sync_info
                if si is not None:
                    waits = list(si.on_wait)
                    if len(waits) > 1:
                        changed = True
                        n_split += 1
                        for j, w in enumerate(waits[:-1]):
                            out.append(
                                mybir.InstNoOp(
                                    name=f"{inst.name}-w{j}",
                                    sync_info=mybir.SyncInfo(
                                        on_wait=[w], on_update=[]
                                    ),
                                    bass_nofuse=True,
                                    engine=inst.engine,
                                )
                            )
                        si.on_wait = waits[-1:]
                out.append(inst)
            if changed:
                blk.instructions = out
    return n_split


def _build_program(legalize=True):
    import concourse.bass as bass
    import concourse.mybir as mybir
    import concourse.tile as tile

    f32 = mybir.dt.float32
    f32r = mybir.dt.float32r
    AF = mybir.ActivationFunctionType
    ALU = mybir.AluOpType
    AX = mybir.AxisListType

    def rx(ap):
        return ap.bitcast(f32r) if RELAXED else ap

    # The BIR verifier requires every producer of an fp32r-matmul operand
    # to emit fp32r-rounded output: write those tensors through fp32r-
    # bitcast views (engines round on the output cast). Non-matmul readers
    # keep plain f32 views of the same bits.
    rw = rx

    nc = bass.Bass()

    def din(name, shape):
        return nc.dram_tensor(name, list(shape), f32, kind="ExternalInput")

    def dout(name, shape):
        return nc.dram_tensor(name, list(shape), f32, kind="ExternalOutput")

    z_d = din("z", [NL, LAT])
    lab_d = din("lab", [NL, 7])
    bc_d = din("BC", [64, EG])
    embw_d = din("embw", [ZIN, HID])
    embb_d = din("embb", [128, 2])
    rp1_d = din("rp1", [ZIN, 15])
    rpb1_d = din("rpb1", [15, 1])
    rp2_d = din("rp2", [15, 3])
    rpb2_d = din("rpb2", [3, 1])
    re1_d = din("re1", [ZIN, 30 * 60])
    reb1_d = din("reb1", [60, 1])
    re2b_d = din("re2b", [61, 4500])
    re2c_d = din("re2c", [61, 4500])
    ones16_d = din("ones16", [1, 128])
    rnb2r_d = din("rnb2r", [1, 5])
    an1_d = din("an1", [ZIN, 30 * 60])
    anb1_d = din("anb1", [60, 1])
    an2_d = din("an2", [60, PAD])
    anb2_d = din("anb2", [PAD, 1])
    idm_d = din("idm", [128, 128])
    rn1_d = din("rn1", [HID, 128])
    big0_d = din("big0", [128, 7448])
    rnb1_d = din("rnb1", [128, 1])
    rn2_d = din("rn2", [128, 5])
    rnb2_d = din("rnb2", [5, 1])

    L = []
    for l in range(N_LAYERS):
        L.append(
            {
                "ehr": din(f"ehr{l}", [HID, HID]),
                "ehc": din(f"ehc{l}", [HID, HID]),
                "wz": din(f"wz{l}", [10, HID]),
                "eb1": din(f"eb1_{l}", [128, 2]),
                "ew2": din(f"ew2_{l}", [HID, HID]),
                "eb2": din(f"eb2_{l}", [128, 2]),
                "cw1": din(f"cw1_{l}", [HID, HID]),
                "cb1": din(f"cb1_{l}", [128, 2]),
                "cw2": din(f"cw2_{l}", [HID, 1]),
                "nw1h": din(f"nw1h_{l}", [HID, HID]),
                "nw1a": din(f"nw1a_{l}", [HID, HID]),
                "nb1": din(f"nb1_{l}", [128, 2]),
                "nw2": din(f"nw2_{l}", [HID, HID]),
                "nb2": din(f"nb2_{l}", [128, 2]),
            }
        )

    hout_d = dout("h_out", [NL, 5])
    coord_d = dout("coord_out", [NL, 3])
    ea_d = dout("ea_out", [EL, 5])
    an_d = dout("an_out", [G, PAD])

    with tile.TileContext(nc) as tc:
        with (
            tc.tile_pool(name="wb", bufs=1) as W,
            tc.tile_pool(name="sb", bufs=2) as SB,
            tc.tile_pool(name="ep", bufs=2, space="PSUM") as EP,
            tc.tile_pool(name="ep2", bufs=5, space="PSUM") as EP2,
            tc.tile_pool(name="wp", bufs=1, space="PSUM") as WP,
        ):
            # ---------------- static loads ----------------
            # Order matters: DMA queues are FIFO, so the tiny inputs that
            # gate the whole dependency tree (z/lab -> z_t transposes ->
            # heads/h/coord) go first, then the weights by first use,
            # with the layer-1 block last.
            idm_t = W.tile([128, 128], f32, tag="idm")
            nc.sync.dma_start(idm_t[:], idm_d.ap())
            z_t = W.tile([ZIN, NL], f32, tag="z_t")
            zls = []
            for c in range(4):
                nsl = slice(c * 120, (c + 1) * 120)
                zl = SB.tile([120, ZIN], f32, tag="zl", name=f"zl{c}")
                nc.sync.dma_start(zl[:, 0:LAT], z_d.ap()[nsl, :])
                nc.scalar.dma_start(zl[:, LAT:ZIN], lab_d.ap()[nsl, :])
                zls.append(zl)
            for c in range(4):
                nsl = slice(c * 120, (c + 1) * 120)
                zps = EP.tile([ZIN, 120], f32, tag="ep", name=f"zps{c}")
                nc.tensor.transpose(zps[:], zls[c][:], idm_t[0:120, 0:120])
                nc.vector.tensor_copy(z_t[:, nsl], zps[:])
            # re2b first: it gates y2_rm -> re6 edge_attr rows -> the very
            # first pre matmul of layer 0 (longest startup pole).
            re2b_t = W.tile([61, 4500], f32, tag="re2b")
            for ci, e in zip(range(3), (nc.scalar, nc.sync, nc.scalar)):
                csl = slice(ci * 1500, (ci + 1) * 1500)
                e.dma_start(rw(re2b_t[:, csl]), rw(re2b_d.ap()[:, csl]))
            big0_t = W.tile([128, 7448], f32, tag="big0")
            nc.sync.dma_start(rw(big0_t[:, 0:1024]), rw(big0_d.ap()[:, 0:1024]))
            nc.scalar.dma_start(
                rw(big0_t[:, 7168:7448]), rw(big0_d.ap()[:, 7168:7448])
            )
            nc.sync.dma_start(
                rw(big0_t[:, 1024:1536]), rw(big0_d.ap()[:, 1024:1536])
            )
            nc.scalar.dma_start(
                rw(big0_t[:, 1536:2048]), rw(big0_d.ap()[:, 1536:2048])
            )
            nc.sync.dma_start(
                rw(big0_t[:, 2048:3584]), rw(big0_d.ap()[:, 2048:3584])
            )
            # zg chunks: zg^T rows [a*71 .. a*71+71) for graph g are just
            # z_t columns g*30+a -> strided slices of z_t, no extra tile.
            zg3 = z_t[:].rearrange("j (g a) -> j a g", a=PAD)
            # Static one-hot broadcast/tile pattern, shared by all graphs.
            # The per-graph radial/edge_attr rows live in a separate
            # [70, 900] tile (rows 64..69) so the pre matmul is
            # K=64 (static) + K=6 (per graph) accumulating in PSUM.
            re6s = []
            for i in range(2):
                r = W.tile([70, EG], f32, tag=f"re6_{i}", name=f"re6s{i}")
                if i == 0:
                    # graph 0's tile gates the first pre matmul: fast
                    # split load on both HWDGE queues
                    nc.sync.dma_start(rw(r[0:32, :]), rw(bc_d.ap()[0:32, :]))
                    nc.scalar.dma_start(rw(r[32:64, :]), rw(bc_d.ap()[32:64, :]))
                else:
                    # off the critical path (first used at graph 1)
                    nc.gpsimd.dma_start(rw(r[0:64, :]), rw(bc_d.ap()))
                re6s.append(r)

            _eng_rot = [nc.sync, nc.scalar, nc.gpsimd]
            _eng_i = [0]

            def _eng():
                e = _eng_rot[_eng_i[0] % len(_eng_rot)]
                _eng_i[0] += 1
                return e

            def wload(d, shape, tag):
                t = W.tile(list(shape), f32, tag=tag)
                _eng().dma_start(t[:], d.ap())
                return t

            embw_t = wload(embw_d, [ZIN, HID], "embw")
            embb_t = wload(embb_d, [128, 2], "embb")
            rp1_t = wload(rp1_d, [ZIN, 15], "rp1")
            rpb1_t = wload(rpb1_d, [15, 1], "rpb1")
            rp2_t = wload(rp2_d, [15, 3], "rp2")
            rpb2_t = wload(rpb2_d, [3, 1], "rpb2")
            re1_t = wload(re1_d, [ZIN, 1800], "re1")
            reb1_t = wload(reb1_d, [60, 1], "reb1")
            re2c_t = W.tile([61, 4500], f32, tag="re2c")
            nc.gpsimd.dma_start(rw(re2c_t[:]), rw(re2c_d.ap()))
            an1_t = wload(an1_d, [ZIN, 1800], "an1")
            anb1_t = wload(anb1_d, [60, 1], "anb1")
            an2_t = wload(an2_d, [60, PAD], "an2")
            anb2_t = wload(anb2_d, [PAD, 1], "anb2")
            rnb1_t = wload(rnb1_d, [128, 1], "rnb1")
            rn2_t = W.tile([128, 5], f32, tag="rn2")
            nc.sync.dma_start(rn2_t[:], rn2_d.ap())
            ones128_t = W.tile([1, 128], f32, tag="ones128")
            nc.scalar.dma_start(ones128_t[:], ones16_d.ap())
            rnb2r_t = W.tile([1, 5], f32, tag="rnb2r")
            nc.scalar.dma_start(rnb2r_t[:], rnb2r_d.ap())
            rn1_t = None  # assigned after the packed load below

            # all [128, x] layer weights packed host-side into one tensor
            # -> a single large contiguous DMA instead of ~32 small ones
            # deferred layer-1 weights (consumed ~halfway through)
            nc.sync.dma_start(
                rw(big0_t[:, 3584:7168]), rw(big0_d.ap()[:, 3584:7168])
            )
            _off = [0]

            def _blk(cols):
                o = _off[0]
                _off[0] += cols
                return big0_t[:, o : o + cols]

            LT = []
            for l in range(N_LAYERS):
                d = L[l]
                t = {}
                for nm in ("ehr", "ehc", "ew2", "cw1", "nw1h", "nw1a", "nw2"):
                    t[nm] = [_blk(HID), _blk(HID)]
                t["wz"] = d["wz"]  # stays in DRAM; DMA'd per graph
                LT.append(t)
            rn1_p = [_blk(128), _blk(128)]
            for l in range(N_LAYERS):
                LT[l]["cw2"] = [_blk(1), _blk(1)]
            for l in range(N_LAYERS):
                for nm in ("eb1", "eb2", "cb1", "nb1", "nb2"):
                    LT[l][nm] = _blk(2)
            rn1_t = rn1_p

            # ---------------- graph-level heads ----------------
            # re_edge hidden: y1 = silu(zg @ W1 + b1), K accumulated in
            # 30 chunks of 71 (one per node slot a).
            y1p = EP.tile([60, G], f32, tag="ep")
            for a in range(PAD):
                nc.tensor.matmul(
                    y1p[:],
                    re1_t[:, a * 60 : (a + 1) * 60],
                    zg3[:, a, :],
                    start=(a == 0),
                    stop=(a == PAD - 1),
                )
            # [y1; ones] so the next matmul folds the bias in
            y1s = SB.tile([61, G], f32, tag="y1s")
            nc.scalar.activation(
                rw(y1s[0:60, :]), y1p[:], AF.Silu, bias=reb1_t[:, 0:1]
            )
            nc.sync.dma_start(rw(y1s[60:61, :]), rw(ones16_d.ap()[:, 0:G]))
            # atom_num hidden
            ay1p = EP.tile([60, G], f32, tag="ep")
            for a in range(PAD):
                nc.tensor.matmul(
                    ay1p[:],
                    an1_t[:, a * 60 : (a + 1) * 60],
                    zg3[:, a, :],
                    start=(a == 0),
                    stop=(a == PAD - 1),
                )
            ay1s = SB.tile([60, G], f32, tag="ay1s")
            nc.scalar.activation(ay1s[:], ay1p[:], AF.Silu, bias=anb1_t[:, 0:1])
            # re_edge out, graph-major with host-permuted columns:
            # y2_rm[g, j*900+k] = edge_attr[g*900+k, j]; bias folded via
            # the ones row of y1s.
            y2_rm = SB.tile([G, 4500], f32, tag="y2_rm", bufs=1)
            for fb in range(9):
                fsl = slice(fb * 500, (fb + 1) * 500)
                y2p = EP.tile([G, 500], f32, tag="ep", name=f"y2p{fb}")
                nc.tensor.matmul(
                    y2p[:], rx(y1s[:]), rx(re2b_t[:, fsl]),
                    start=True, stop=True,
                )
                nc.vector.tensor_copy(y2_rm[:, fsl], y2p[:])
            # edge_attr output: second head pass with unpermuted columns
            # so the DRAM write is fully contiguous (16 descriptors/block)
            ea2d = ea_d.ap().rearrange("e j -> (e j)").rearrange(
                "(g f) -> g f", g=G
            )
            for fb in range(9):
                fsl = slice(fb * 500, (fb + 1) * 500)
                eap = EP.tile([G, 500], f32, tag="ep", name=f"eap{fb}")
                nc.tensor.matmul(
                    eap[:], rx(y1s[:]), rx(re2c_t[:, fsl]),
                    start=True, stop=True,
                )
                eas = SB.tile([G, 500], f32, tag="eas", bufs=2, name=f"eas{fb}")
                nc.vector.tensor_copy(eas[:], eap[:])
                nc.sync.dma_start(ea2d[:, fsl], eas[:])
            # atom_num out
            ayp = EP.tile([PAD, G], f32, tag="ep")
            nc.tensor.matmul(ayp[:], an2_t[:], ay1s[:], start=True, stop=True)
            ay_sb = SB.tile([PAD, G], f32, tag="ay_sb")
            nc.scalar.activation(
                ay_sb[:], ayp[:], AF.Identity, bias=anb2_t[:, 0:1]
            )
            nc.sync.dma_start(
                an_d.ap().rearrange("g k -> k g"), ay_sb[:]
            )


            # ---------------- h / coord init ----------------
            h_cur = []
            for fb in range(2):
                hp = EP.tile([128, NL], f32, tag="ep")
                nc.tensor.matmul(
                    hp[:], embw_t[:, fb * 128 : (fb + 1) * 128], z_t[:],
                    start=True, stop=True,
                )
                ht = SB.tile([128, NL], f32, tag=f"h{fb}", bufs=1)
                nc.scalar.activation(
                    rw(ht[:]), hp[:], AF.Identity, bias=embb_t[:, fb : fb + 1]
                )
                h_cur.append(ht)
            cp1 = EP.tile([15, NL], f32, tag="ep")
            nc.tensor.matmul(cp1[:], rp1_t[:], z_t[:], start=True, stop=True)
            c1s = SB.tile([15, NL], f32, tag="c1s")
            nc.scalar.activation(c1s[:], cp1[:], AF.Silu, bias=rpb1_t[:, 0:1])
            cp2 = EP.tile([3, NL], f32, tag="ep")
            nc.tensor.matmul(cp2[:], rp2_t[:], c1s[:], start=True, stop=True)
            coordT = SB.tile([3, NL], f32, tag="coordT")
            nc.scalar.activation(
                coordT[:], cp2[:], AF.Identity, bias=rpb2_t[:, 0:1]
            )
            # coordS[g, d*30+a] = coord[g*30+a, d]
            coordS = SB.tile([G, 3 * PAD], f32, tag="coordS", bufs=1)
            for d in range(3):
                # shapes differ but iteration orders match: (g, a) vs g*30+a
                nc.sync.dma_start(
                    coordS[:, d * PAD : (d + 1) * PAD],
                    coordT[d : d + 1, :],
                )

            # ---------------- message-passing layers ----------------
            for l in range(N_LAYERS):
                t = LT[l]
                # coordinate differences, stacked graphs on partitions
                cd = []
                for d in range(3):
                    cdt = SB.tile([G, EG], f32, tag=f"cd{d}", bufs=1)
                    ca = coordS[:, d * PAD : (d + 1) * PAD]
                    nc.vector.tensor_sub(
                        cdt[:].rearrange("g (a b) -> g a b", b=PAD),
                        ca.broadcast_to([G, PAD, PAD]),
                        ca[:, None, :].broadcast_to([G, PAD, PAD]),
                    )
                    cd.append(cdt)
                radial = SB.tile([G, EG], f32, tag="radial", bufs=1)
                sqt = SB.tile([G, EG], f32, tag="trans", bufs=1)
                nc.vector.tensor_mul(radial[:], cd[0][:], cd[0][:])
                nc.vector.tensor_mul(sqt[:], cd[1][:], cd[1][:])
                nc.vector.tensor_add(radial[:], radial[:], sqt[:])
                nc.vector.tensor_mul(sqt[:], cd[2][:], cd[2][:])
                nc.vector.tensor_add(radial[:], radial[:], sqt[:])
                agg = [
                    SB.tile([128, NL], f32, tag=f"agg{fb}", bufs=1, name=f"agg{l}_{fb}")
                    for fb in range(2)
                ]
                w_all = SB.tile([G, EG], f32, tag="w_all", bufs=1)

                for g in range(G):
                    ns = slice(g * PAD, (g + 1) * PAD)
                    # P and Q node-level pre-products, both M=30 at (0,0)
                    # (the fp32r ISA check rejects col-offset tile_position,
                    # so Q reaches s70 rows 32..61 via a staging DMA).
                    psP = EP.tile([30, HID], f32, tag="ep", name=f"psP{l}_{g}")
                    for kc in range(2):
                        nc.tensor.matmul(
                            psP[:],
                            rx(h_cur[kc][:, ns]),
                            rx(t["ehr"][kc][:]),
                            start=(kc == 0),
                            stop=(kc == 1),
                        )
                    psQ = EP.tile([30, HID], f32, tag="ep", name=f"psQ{l}_{g}")
                    for kc in range(2):
                        nc.tensor.matmul(
                            psQ[:],
                            rx(h_cur[kc][:, ns]),
                            rx(t["ehc"][kc][:]),
                            start=(kc == 0),
                            stop=(kc == 1),
                        )
                    s70 = SB.tile([70, HID], f32, tag="s70", bufs=4)
                    nc.sync.dma_start(rw(s70[30:32, :]), rw(t["wz"].ap()[0:2, :]))
                    nc.sync.dma_start(rw(s70[62:70, :]), rw(t["wz"].ap()[2:10, :]))
                    nc.vector.tensor_copy(rw(s70[0:30, :]), psP[:])
                    qst = SB.tile([30, HID], f32, tag="qst", bufs=3, name=f"qst{l}_{g}")
                    nc.vector.tensor_copy(rw(qst[:]), psQ[:])
                    nc.sync.dma_start(rw(s70[32:62, :]), rw(qst[:]))
                    # per-graph radial + edge_attr rows at partitions
                    # 64..69 of the static ping-pong tile (BC rows 0..63
                    # were filled once at setup -> single K=70 pre matmul)
                    re6 = re6s[g % 2]
                    nc.sync.dma_start(rw(re6[64:65, :]), rw(radial[g : g + 1, :]))
                    for j in range(5):
                        nc.sync.dma_start(
                            rw(re6[65 + j : 66 + j, :]),
                            rw(y2_rm[g : g + 1, j * 900 : (j + 1) * 900]),
                        )

                    wv_sb = SB.tile([1, EG], f32, tag="wv_sb", bufs=1, name=f"wv_sb{l}_{g}")
                    for hh in range(2):
                        cs = slice(hh * 450, (hh + 1) * 450)
                        # edge MLP stage 1 (pre = e_in @ e_W1)
                        m1 = []
                        for fb in range(2):
                            fs = slice(fb * 128, (fb + 1) * 128)
                            pre = EP2.tile([128, 450], f32, tag="ep2")
                            nc.tensor.matmul(
                                pre[:], rx(s70[:, fs]), rx(re6[:, cs]),
                                start=True, stop=True,
                            )
                            m1t = SB.tile([128, 450], f32, tag=f"m1_{fb}", bufs=4)
                            nc.scalar.activation(
                                rw(m1t[:]), pre[:], AF.Silu,
                                bias=t["eb1"][:, fb : fb + 1],
                            )
                            m1.append(m1t)
                        # edge MLP stage 2 (m = silu(m1 @ e_W2 + b2))
                        mg = []
                        for fb in range(2):
                            fs = slice(fb * 128, (fb + 1) * 128)
                            pre2 = EP2.tile([128, 450], f32, tag="ep2")
                            for kc in range(2):
                                nc.tensor.matmul(
                                    pre2[:], rx(t["ew2"][kc][:, fs]),
                                    rx(m1[kc][:]),
                                    start=(kc == 0), stop=(kc == 1),
                                )
                            mt = SB.tile([128, 450], f32, tag=f"m_{fb}", bufs=4)
                            nc.scalar.activation(
                                rw(mt[:]), pre2[:], AF.Silu,
                                bias=t["eb2"][:, fb : fb + 1],
                            )
                            mg.append(mt)
                        # coord gate hidden
                        mw = []
                        for fb in range(2):
                            fs = slice(fb * 128, (fb + 1) * 128)
                            prew = EP2.tile([128, 450], f32, tag="ep2")
                            for kc in range(2):
                                nc.tensor.matmul(
                                    prew[:], rx(t["cw1"][kc][:, fs]),
                                    rx(mg[kc][:]),
                                    start=(kc == 0), stop=(kc == 1),
                                )
                            mwt = SB.tile([128, 450], f32, tag=f"mw_{fb}")
                            nc.scalar.activation(
                                rw(mwt[:]), prew[:], AF.Silu,
                                bias=t["cb1"][:, fb : fb + 1],
                            )
                            mw.append(mwt)
                        # w = mw @ c_W2 -> [1, 450] (one psum bank per half)
                        wv = WP.tile([1, 450], f32, tag="wp", name=f"wv{l}_{g}_{hh}")
                        for kc in range(2):
                            nc.tensor.matmul(
                                wv[:], rx(t["cw2"][kc][:]), rx(mw[kc][:]),
                                start=(kc == 0), stop=(kc == 1),
                            )
                        nc.vector.tensor_copy(wv_sb[:, cs], wv[:])
                        # agg[f, a] += over this half's 15 a-rows
                        for fb in range(2):
                            with nc.allow_low_precision("f32r agg feeds f32r matmul"):
                                nc.vector.tensor_reduce(
                                    rw(agg[fb][:, g * PAD + hh * 15 : g * PAD + (hh + 1) * 15]),
                                    mg[fb][:].rearrange("p (a b) -> p a b", b=PAD),
                                    axis=AX.X,
                                    op=ALU.add,
                                )
                    # DVE cannot shift partitions: wv_sb staged on partition
                    # 0, DMA moves it to w_all row g.
                    nc.sync.dma_start(w_all[g : g + 1, :], wv_sb[:])

                # coord update: coordS += (sum_b cd*w) / 30
                for d in range(3):
                    trans = SB.tile([G, EG], f32, tag="trans", bufs=1, name=f"trans{l}_{d}")
                    nc.vector.tensor_mul(trans[:], cd[d][:], w_all[:])
                    ssum = SB.tile([G, PAD], f32, tag="ssum", bufs=1, name=f"ssum{l}_{d}")
                    nc.vector.tensor_reduce(
                        ssum[:],
                        trans[:].rearrange("g (a b) -> g a b", b=PAD),
                        axis=AX.X,
                        op=ALU.add,
                    )
                    nc.vector.scalar_tensor_tensor(
                        coordS[:, d * PAD : (d + 1) * PAD],
                        ssum[:],
                        1.0 / PAD,
                        coordS[:, d * PAD : (d + 1) * PAD],
                        op0=ALU.mult,
                        op1=ALU.add,
                    )

                # node update
                s1 = []
                for fb in range(2):
                    hn = EP.tile([128, NL], f32, tag="ep")
                    for kc in range(2):
                        nc.tensor.matmul(
                            hn[:],
                            rx(t["nw1h"][kc][:, fb * 128 : (fb + 1) * 128]),
                            rx(h_cur[kc][:]),
                            start=(kc == 0),
                            stop=False,
                        )
                    for kc in range(2):
                        nc.tensor.matmul(
                            hn[:],
                            rx(t["nw1a"][kc][:, fb * 128 : (fb + 1) * 128]),
                            rx(agg[kc][:]),
                            start=False,
                            stop=(kc == 1),
                        )
                    s1t = SB.tile([128, NL], f32, tag=f"s1_{fb}", bufs=2)
                    nc.scalar.activation(
                        rw(s1t[:]), hn[:], AF.Silu, bias=t["nb1"][:, fb : fb + 1]
                    )
                    s1.append(s1t)
                # h never materializes: every consumer of h (next layer's
                # P/Q + node matmuls, or the h_out head) is linear, so
                # n_W2/n_b2 are composed into those weights host-side and
                # the next stage reads s1 directly.
                h_cur = s1

            # ---------------- output heads ----------------
            # coord first so its (scattered) store overlaps the h_out head
            coord3 = coord_d.ap().rearrange("(g a) d -> g d a", a=PAD)
            for d, e in zip(range(3), (nc.sync, nc.scalar, nc.gpsimd)):
                e.dma_start(
                    coord3[:, d, :], coordS[:, d * PAD : (d + 1) * PAD]
                )
            r1p = EP.tile([128, NL], f32, tag="ep")
            for kc in range(2):
                nc.tensor.matmul(
                    r1p[:], rx(rn1_t[kc][:]), rx(h_cur[kc][:]),
                    start=(kc == 0), stop=(kc == 1),
                )
            r1s = SB.tile([128, NL], f32, tag="r1s")
            nc.scalar.activation(r1s[:], r1p[:], AF.Silu, bias=rnb1_t[:, 0:1])
            # node-major blocks: contiguous h_out rows, bias via ones row
            for c in range(4):
                nbl = slice(c * 120, (c + 1) * 120)
                hop = EP.tile([120, 5], f32, tag="ep", name=f"hop{c}")
                nc.tensor.matmul(
                    hop[:], r1s[:, nbl], rn2_t[:],
                    start=True, stop=False,
                )
                nc.tensor.matmul(
                    hop[:], ones128_t[:, 0:120], rnb2r_t[:],
                    start=False, stop=True,
                )
                hos = SB.tile([120, 5], f32, tag="hos", bufs=2, name=f"hos{c}")
                nc.vector.tensor_copy(hos[:], hop[:])
                nc.sync.dma_start(hout_d.ap()[nbl, :], hos[:])

    import concourse.mybir as mybir  # noqa: F811

    if legalize:
        _legalize_waits(nc, mybir)
    return nc


def _prep_weights(params):
    """Host-side weight massaging -> dict of named f32 arrays."""
    def A(x):
        return np.ascontiguousarray(np.asarray(x), dtype=np.float32)

    def col2(b):  # [256] -> [128, 2], column fb = features fb*128..
        return np.ascontiguousarray(A(b).reshape(2, 128).T)

    p = {}
    p["embw"] = A(params["emb_W"])
    p["embb"] = col2(params["emb_b"])
    rp = params["re_pos"]
    p["rp1"] = A(rp["W1"])
    p["rpb1"] = A(rp["b1"]).reshape(15, 1)
    p["rp2"] = A(rp["W2"])
    p["rpb2"] = A(rp["b2"]).reshape(3, 1)
    re = params["re_edge"]
    p["re1"] = np.ascontiguousarray(
        A(re["W1"]).reshape(PAD, ZIN, 60).transpose(1, 0, 2).reshape(ZIN, PAD * 60)
    )
    p["reb1"] = A(re["b1"]).reshape(60, 1)
    W2p = A(re["W2"]).reshape(60, 900, 5).transpose(0, 2, 1).reshape(60, 4500)
    b2p = A(re["b2"]).reshape(900, 5).T.reshape(1, 4500)
    p["re2b"] = np.ascontiguousarray(np.concatenate([W2p, b2p], axis=0))
    p["re2c"] = np.ascontiguousarray(
        np.concatenate([A(re["W2"]), A(re["b2"]).reshape(1, 4500)], axis=0)
    )
    p["ones16"] = np.ones((1, 128), dtype=np.float32)
    an = params["atom_num"]
    p["an1"] = np.ascontiguousarray(
        A(an["W1"]).reshape(PAD, ZIN, 60).transpose(1, 0, 2).reshape(ZIN, PAD * 60)
    )
    p["anb1"] = A(an["b1"]).reshape(60, 1)
    p["an2"] = A(an["W2"])
    p["anb2"] = A(an["b2"]).reshape(PAD, 1)
    rn = params["re_nodes"]
    p["idm"] = np.eye(128, dtype=np.float32)
    p["rn1"] = A(rn["W1"])
    lpL0 = params["layers"][-1]
    rnb1c = A(rn["b1"]).astype(np.float64) + A(lpL0["n_b2"]).astype(
        np.float64
    ) @ A(rn["W1"]).astype(np.float64)
    p["rnb1"] = rnb1c.astype(np.float32).reshape(128, 1)
    p["rn2"] = A(rn["W2"])
    p["rnb2"] = A(rn["b2"]).reshape(5, 1)
    p["rnb2r"] = A(rn["b2"]).reshape(1, 5)

    # static one-hot broadcast/tile pattern
    bc = np.zeros((64, EG), dtype=np.float32)
    for a in range(PAD):
        bc[a, a * PAD : (a + 1) * PAD] = 1.0
    for b in range(PAD):
        bc[32 + b, b::PAD] = 1.0
    p["BC"] = bc

    blocks = []
    lp0 = params["layers"][0]
    nW2_0 = A(lp0["n_W2"]).astype(np.float64)
    nb2_0 = A(lp0["n_b2"]).astype(np.float64)
    for li, lp in enumerate(params["layers"]):
        eW1 = A(lp["e_W1"])
        nW1 = A(lp["n_W1"])
        ehr_w = eW1[0:256]
        ehc_w = eW1[256:512]
        nw1h_w = nW1[0:256] + nW1[512:768]
        if li == 1:
            # layer-1 inputs read s1_0: fold layer-0's output projection in
            ehr_w = (nW2_0 @ ehr_w.astype(np.float64)).astype(np.float32)
            ehc_w = (nW2_0 @ ehc_w.astype(np.float64)).astype(np.float32)
            nw1h_w = (nW2_0 @ nw1h_w.astype(np.float64)).astype(np.float32)
        for arr in (
            ehr_w,
            ehc_w,
            A(lp["e_W2"]),
            A(lp["c_W1"]),
            nw1h_w,
            nW1[256:512],
            A(lp["n_W2"]),
        ):
            blocks.append(arr[0:128, :])
            blocks.append(arr[128:256, :])
    lpL = params["layers"][-1]
    rn_w1 = A(rn["W1"])
    rnc = (A(lpL["n_W2"]).astype(np.float64) @ rn_w1.astype(np.float64)).astype(
        np.float32
    )
    blocks.append(rnc[0:128, :])
    blocks.append(rnc[128:256, :])
    for lp in params["layers"]:
        cw2 = A(lp["c_W2"]).reshape(HID, 1)
        blocks.append(cw2[0:128, :])
        blocks.append(cw2[128:256, :])
    for li, lp in enumerate(params["layers"]):
        eb1_v = A(lp["e_b1"]).astype(np.float64)
        nb1_v = A(lp["n_b1"]).astype(np.float64)
        if li == 1:
            eW1f = A(lp["e_W1"]).astype(np.float64)
            eb1_v = eb1_v + nb2_0 @ (eW1f[0:256] + eW1f[256:512])
            nW1f = A(lp["n_W1"]).astype(np.float64)
            nb1_v = nb1_v + nb2_0 @ (nW1f[0:256] + nW1f[512:768])
        for b in (
            eb1_v.astype(np.float32),
            A(lp["e_b2"]),
            A(lp["c_b1"]),
            nb1_v.astype(np.float32),
            A(lp["n_b2"]),
        ):
            blocks.append(col2(b))
    p["big0"] = np.ascontiguousarray(np.concatenate(blocks, axis=1))
    assert p["big0"].shape == (128, 7448), p["big0"].shape

    for l, lp in enumerate(params["layers"]):
        eW1 = A(lp["e_W1"])  # [518, 256]
        p[f"ehr{l}"] = np.ascontiguousarray(eW1[0:256])
        p[f"ehc{l}"] = np.ascontiguousarray(eW1[256:512])
        wz = np.zeros((10, HID), dtype=np.float32)
        wz[4] = eW1[512]
        wz[5:10] = eW1[513:518]
        p[f"wz{l}"] = wz
        p[f"eb1_{l}"] = col2(lp["e_b1"])
        p[f"ew2_{l}"] = A(lp["e_W2"])
        p[f"eb2_{l}"] = col2(lp["e_b2"])
        p[f"cw1_{l}"] = A(lp["c_W1"])
        p[f"cb1_{l}"] = col2(lp["c_b1"])
        p[f"cw2_{l}"] = A(lp["c_W2"]).reshape(HID, 1)
        nW1 = A(lp["n_W1"])  # [768, 256]
        p[f"nw1h_{l}"] = np.ascontiguousarray(nW1[0:256] + nW1[512:768])
        p[f"nw1a_{l}"] = np.ascontiguousarray(nW1[256:512])
        p[f"nb1_{l}"] = col2(lp["n_b1"])
        p[f"nw2_{l}"] = A(lp["n_W2"])
        p[f"nb2_{l}"] = col2(lp["n_b2"])
    return p


def _expected_edges():
    i = np.arange(PAD)
    row = np.repeat(i, PAD)
    col = np.tile(i, PAD)
    offs = (np.arange(B) * PAD)[:, None]
    return np.stack(
        [
            (row[None, :] + offs).reshape(-1),
            (col[None, :] + offs).reshape(-1),
        ]
    ).astype(np.int64)


def _get_nc():
    if "nc" not in _CACHE:
        _CACHE["nc"] = _build_program()
    return _CACHE["nc"]


def _install_ntff_hook():
    """antenv.axon_hooks is absent in this image; shim it and register the
    ctypes NTFF profiling hook from trn_agent_boot so trace=True works."""
    import types
    import importlib

    try:
        importlib.import_module("antenv.axon_hooks")
        return
    except ImportError:
        pass
    try:
        from trn_agent_boot.trn_boot import _ntff_profile_via_ctypes
    except ImportError:
        return
    import antenv

    mod = types.ModuleType("antenv.axon_hooks")
    mod._hook = _ntff_profile_via_ctypes("/opt/axon/libaxon_pjrt.so")

    def set_axon_ntff_profile_hook(h):
        mod._hook = h

    def get_axon_ntff_profile_hook():
        return mod._hook

    mod.set_axon_ntff_profile_hook = set_axon_ntff_profile_hook
    mod.get_axon_ntff_profile_hook = get_axon_ntff_profile_hook
    sys.modules["antenv.axon_hooks"] = mod
    antenv.axon_hooks = mod


def kernel(z, label, edges, n_nodes, params, _trace=False):
    from concourse.bass_utils import run_bass_kernel_spmd

    z = np.asarray(z, dtype=np.float32)
    label = np.asarray(label, dtype=np.float32)
    edges_np = np.asarray(edges)
    assert int(n_nodes) == PAD, f"n_nodes must be {PAD}"
    if not np.array_equal(edges_np.astype(np.int64), _expected_edges()):
        raise ValueError("edges do not match the full per-graph grid layout")

    wmap = _prep_weights(params)
    nc = _get_nc()

    in_maps = []
    for c in range(NCORES):
        ns = slice(c * NL, (c + 1) * NL)
        m = dict(wmap)
        m["z"] = np.ascontiguousarray(z[ns])
        m["lab"] = np.ascontiguousarray(label[ns])
        in_maps.append(m)

    kw = {}
    if _trace:
        _install_ntff_hook()
        kw = dict(trace=True, trace_cores=[0])
    res = run_bass_kernel_spmd(nc, in_maps, core_ids=list(range(NCORES)), **kw)

    h_out = np.concatenate([res.results[c]["h_out"] for c in range(NCORES)], 0)
    coord = np.concatenate(
        [res.results[c]["coord_out"] for c in range(NCORES)], 0
    )
    ea = np.concatenate([res.results[c]["ea_out"] for c in range(NCORES)], 0)
    an = np.concatenate([res.results[c]["an_out"] for c in range(NCORES)], 0)
    if _trace:
        return (h_out, coord, ea, an), res
    return h_out, coord, ea, an
